# revision 5
# baseline (speedup 1.0000x reference)
"""Trainium2 Bass kernel for GQA attention with QK-RMSNorm, RoPE and a
bidirectional-prefix + causal mask (sparse_attention problem).

Reference computation (fp32):
  xq = x @ wq.T; xk = x @ wk.T; xv = x @ wv.T   (per-head RMSNorm on q,k)
  rope(q), rope(k); repeat kv heads 8x
  scores = q k^T / sqrt(128); mask = causal OR (i<p & j<p)
  out = softmax(scores) @ v;  y = out @ wo.T

Sharding: 8 cores = 2 batches x 4 head-groups (4 query heads each, sharing
one KV head).  Each core computes a partial y^T (its 4 heads' contribution);
the host sums the 4 partials per batch and transposes back.

v2 design notes (vs the first working version):
  * No fp32->fp32r staging copies: matmul-consumed DRAM tensors and SBUF
    tiles are declared float32r (same 32-bit layout host-side) so DMA
    lands them directly; engine-written operands write f32r natively.
  * RMSNorm sum-of-squares runs on the Scalar engine (Square + accum_out),
    the per-token 1/sqrt scale is applied FOR FREE by the PE transpose:
    instead of transposing with the identity, we transpose with
    diag(rq) so qT = q.T @ diag(rq) lands pre-scaled.
  * RoPE sign is folded into sin (host negates the low half), so rope is
    3 wide DVE multiplies + 1 add across all 4 heads at once, reading the
    projection results directly from PSUM.
  * K-path rope + the diag(rq) builds run on the otherwise-idle GPSIMD.
  * Softmax: exp on ACT (per 2-block pair), row sums via a ones-matmul,
    reciprocal on DVE, and the [1,512] -> [128,512] broadcast is a rank-1
    PE matmul (ones outer product) instead of a DRAM round-trip.
  * Single PE instruction stream ordered so the PE never has a long gap
    (projection MMs -> previous tb's transposes -> ... -> attention),
    keeping the HAM clock-gate at 8/8.
  * ACT program order is strictly {Square,Copy,Sqrt} then {Exp,Copy}, so
    exactly two activation-table loads happen.

TRN2 ISA allows ONE sync-wait per instruction and walrus does not split
multi-wait instructions, so `_legalize_waits` rewrites the emitted BIR,
moving excess waits onto preceding same-engine NoOps.
"""
import math
import numpy as np
from contextlib import ExitStack

import bass_rust
import concourse.bass as bass
import concourse.mybir as mybir
import concourse.tile as tile
from concourse.bass_utils import run_bass_kernel_spmd
from concourse.masks import make_identity

F32 = mybir.dt.float32
F32R = mybir.dt.float32r
AF = mybir.ActivationFunctionType

B, S, D = 2, 2048, 2048
NH, KVH, HD = 16, 2, 128
HPC = 4                      # query heads per core
N_CORES = 8
EPS = 1e-6
SOFT_SCALE = 1.0 / math.sqrt(HD)
NEG = -1.0e30

SB = S // 128                # 16 token blocks
DB = D // 128                # 16 contraction blocks

_lgw_counter = [0]


def _legalize_waits(nc, cap=1):
    """Move all-but-`cap` sync waits of every instruction onto preceding
    same-engine NoOps (TRN2 EVENTS block has a single wait slot)."""
    for fn in nc.m.functions:
        for blk in fn.blocks:
            out = []
            changed = False
            for inst in blk.instructions:
                si = inst.sync_info
                waits = list(si.on_wait) if si is not None and si.on_wait else []
                if len(waits) > cap:
                    changed = True
                    move, keep = waits[:-cap], waits[-cap:]
                    for w in move:
                        n = bass_rust.InstNoOp(name=f"LGW-{_lgw_counter[0]}")
                        _lgw_counter[0] += 1
                        n.engine = inst.engine
                        n.sync_info = mybir.SyncInfo(on_wait=[w], on_update=[])
                        out.append(n)
                    inst.sync_info = mybir.SyncInfo(
                        on_wait=keep, on_update=list(si.on_update or []))
                out.append(inst)
            if changed:
                blk.instructions = out
    return nc


def _ext(rb, p):
    """Key extent attended by query row-block rb (rows rb*128 .. rb*128+127)."""
    lo, hi = rb * 128, (rb + 1) * 128
    if hi <= p:
        return p              # prefix rows attend the full prefix [0, p)
    return hi                 # causal rows attend [0, hi), diag-masked


def build_core_kernel(p, legalize=True):
    """One SPMD program; per-core behavior differs only via input data."""
    nc = bass.Bass()

    xT = nc.dram_tensor("xT", [D, S], F32R, kind="ExternalInput")
    wqT = nc.dram_tensor("wqT", [D, HPC * HD], F32R, kind="ExternalInput")
    wkvT = nc.dram_tensor("wkvT", [D, 2 * HD], F32R, kind="ExternalInput")
    woT = nc.dram_tensor("woT", [HPC * HD, D], F32R, kind="ExternalInput")
    cos_q = nc.dram_tensor("cos_q", [S, HD], F32, kind="ExternalInput")
    sin_q = nc.dram_tensor("sin_q", [S, HD], F32, kind="ExternalInput")
    cos_k = nc.dram_tensor("cos_k", [S, HD], F32, kind="ExternalInput")
    sin_k = nc.dram_tensor("sin_k", [S, HD], F32, kind="ExternalInput")
    dmask = nc.dram_tensor("dmask", [128, 128], F32, kind="ExternalInput")
    yT = nc.dram_tensor("yT", [D, S], F32, kind="ExternalOutput")

    with tile.TileContext(nc) as tc, ExitStack() as octx:
        const = octx.enter_context(tc.tile_pool(name="const", bufs=1))
        ident = const.tile([128, 128], F32)
        make_identity(nc, ident)
        dmask_sb = const.tile([128, 128], F32)
        nc.sync.dma_start(out=dmask_sb, in_=dmask[:, :])
        eps_t = const.tile([128, 1], F32)
        nc.vector.memset(eps_t, EPS)
        ones_f = const.tile([128, 1], F32)
        nc.vector.memset(ones_f, 1.0)
        ones_col = const.tile([128, 1], F32R)
        nc.vector.tensor_copy(out=ones_col, in_=ones_f)
        ones_rf = const.tile([1, 128], F32)
        nc.vector.memset(ones_rf, 1.0)
        ones_row = const.tile([1, 128], F32R)
        nc.vector.tensor_copy(out=ones_row, in_=ones_rf)

        qkv = octx.enter_context(tc.tile_pool(name="qkv", bufs=1))
        qT_all = qkv.tile([128, HPC, S], F32R)        # [hd, h, tok]
        kT_all = qkv.tile([128, S], F32R)             # [hd, tok]
        v_all = qkv.tile([128, SB, HD], F32R)         # [tok(P), tb, hd]

        wpool = octx.enter_context(tc.tile_pool(name="w", bufs=1))
        wq_sb = wpool.tile([128, DB, HPC * HD], F32R)
        wkv_sb = wpool.tile([128, DB, 2 * HD], F32R)
        wo_sb = wpool.tile([128, HPC, D], F32R)

        # weight DMAs: wq first (q matmuls need it first), then wkv;
        # wo is deferred into the tb loop so x/cos DMAs aren't starved.
        for kb in range(DB):
            nc.sync.dma_start(out=wq_sb[:, kb, :],
                              in_=wqT[kb * 128:(kb + 1) * 128, :])
        for kb in range(DB):
            nc.sync.dma_start(out=wkv_sb[:, kb, :],
                              in_=wkvT[kb * 128:(kb + 1) * 128, :])

        # ---------------- Phase 1: QKV projections + norm/rope -------------
        with tc.tile_pool(name="xp", bufs=3) as xp, \
             tc.tile_pool(name="cs", bufs=2) as cs, \
             tc.tile_pool(name="rp", bufs=2) as rp, \
             tc.tile_pool(name="dg", bufs=10) as dgp, \
             tc.tile_pool(name="qps", bufs=2, space="PSUM") as qps_pool, \
             tc.tile_pool(name="kvps", bufs=2, space="PSUM") as kvps_pool, \
             tc.tile_pool(name="trps", bufs=4, space="PSUM") as trps:

            pend_tr = []          # deferred transpose emissions

            def flush_tr():
                for emit in pend_tr:
                    emit()
                del pend_tr[:]

            for tb in range(SB):
                ts = slice(tb * 128, (tb + 1) * 128)

                x_h = []
                for half in range(2):
                    xh = xp.tile([128, 8, 128], F32R, tag=f"x{half}")
                    nc.sync.dma_start(
                        out=xh,
                        in_=xT[half * 1024:(half + 1) * 1024, ts]
                        .rearrange("(kb pp) t -> pp kb t", pp=128))
                    x_h.append(xh)
                cq = cs.tile([128, HD], F32, tag="cq")
                nc.sync.dma_start(out=cq, in_=cos_q[ts, :])
                sq = cs.tile([128, HD], F32, tag="sq")
                nc.sync.dma_start(out=sq, in_=sin_q[ts, :])
                ck = cs.tile([128, HD], F32, tag="ck")
                nc.sync.dma_start(out=ck, in_=cos_k[ts, :])
                sk = cs.tile([128, HD], F32, tag="sk")
                nc.sync.dma_start(out=sk, in_=sin_k[ts, :])
                if tb == 0:
                    for hb in range(HPC):
                        nc.sync.dma_start(
                            out=wo_sb[:, hb, :],
                            in_=woT[hb * 128:(hb + 1) * 128, :])

                q_ps = qps_pool.tile([128, HPC * HD], F32, tag="q")
                kv_ps = kvps_pool.tile([128, 2 * HD], F32, tag="kv")
                for kb in range(DB):
                    xb = x_h[kb // 8][:, kb % 8, :]
                    nc.tensor.matmul(q_ps, lhsT=xb, rhs=wq_sb[:, kb, :],
                                     start=(kb == 0), stop=(kb == DB - 1))
                    nc.tensor.matmul(kv_ps, lhsT=xb, rhs=wkv_sb[:, kb, :],
                                     start=(kb == 0), stop=(kb == DB - 1))
                # previous tb's transposes keep the PE busy while this tb's
                # rope/rms runs on DVE/ACT/GPSIMD.
                flush_tr()

                # --- RMS stats on ACT (Square accumulates sum along free) ---
                ms = rp.tile([128, 8], F32, tag="ms")
                scr = rp.tile([128, HD], F32, tag="scr")
                for h in range(HPC):
                    nc.scalar.activation(
                        out=scr, in_=q_ps[:, h * HD:(h + 1) * HD],
                        func=AF.Square, accum_out=ms[:, h:h + 1])
                k_sb = rp.tile([128, HD], F32, tag="k_sb")
                nc.scalar.copy(out=k_sb, in_=kv_ps[:, 0:HD])
                nc.scalar.copy(out=v_all[:, tb, :], in_=kv_ps[:, HD:])
                nc.scalar.activation(out=scr, in_=k_sb, func=AF.Square,
                                     accum_out=ms[:, HPC:HPC + 1])
                srq = rp.tile([128, 8], F32, tag="srq")
                nc.scalar.activation(out=srq[:, 0:HPC + 1],
                                     in_=ms[:, 0:HPC + 1], func=AF.Sqrt,
                                     bias=eps_t, scale=1.0 / HD)
                rq = rp.tile([128, 8], F32, tag="rq")
                nc.vector.reciprocal(out=rq[:, 0:HPC + 1],
                                     in_=srq[:, 0:HPC + 1])

                # --- rope q: wide over all 4 heads, straight from PSUM ---
                # cos/sin broadcast across heads via stride-0 views
                h2 = HD // 2
                cq4 = bass.AP(tensor=cq.tensor, offset=cq.offset,
                              ap=[list(cq.ap[0]), [0, HPC]] + list(cq.ap[1:]))
                sq4 = bass.AP(tensor=sq.tensor, offset=sq.offset,
                              ap=[list(sq.ap[0]), [0, HPC]] + list(sq.ap[1:]))
                q4 = q_ps[:, :].rearrange("pp (h d) -> pp h d", h=HPC)
                t14 = rp.tile([128, HPC, HD], F32, tag="t14")
                nc.vector.tensor_mul(t14, q4, cq4)
                t2p4 = rp.tile([128, HPC, HD], F32, tag="t2p4")
                # low half: q_hi * (-sin_lo)  (sign folded on host)
                nc.vector.tensor_mul(
                    t2p4[:, :, 0:h2], q4[:, :, h2:HD], sq4[:, :, 0:h2])
                nc.vector.tensor_mul(
                    t2p4[:, :, h2:HD], q4[:, :, 0:h2], sq4[:, :, h2:HD])
                qr = rp.tile([128, HPC, HD], F32R, tag="qr")
                nc.vector.tensor_add(qr, t14, t2p4)

                # --- rope k on GPSIMD (SBUF-only engine) ---
                kr = rp.tile([128, HD], F32R, tag="kr")
                t1k = rp.tile([128, HD], F32, tag="t1k")
                nc.gpsimd.tensor_mul(t1k, k_sb, ck)
                t2k = rp.tile([128, HD], F32, tag="t2k")
                nc.gpsimd.tensor_mul(t2k[:, 0:h2], k_sb[:, h2:HD], sk[:, 0:h2])
                nc.gpsimd.tensor_mul(t2k[:, h2:HD], k_sb[:, 0:h2], sk[:, h2:HD])
                nc.gpsimd.tensor_add(kr, t1k, t2k)

                # --- diag(rq) tiles on GPSIMD ---
                dgs = []
                for j in range(HPC + 1):
                    dg = dgp.tile([128, 128], F32R, tag=f"dg{j}")
                    nc.gpsimd.tensor_scalar_mul(dg, ident, rq[:, j:j + 1])
                    dgs.append(dg)

                # --- transposes (deferred into next tb's PE slot):
                #     qT = qr.T @ diag(rq) applies the RMS scale for free ---
                def emit_tr(tb=tb, ts=ts, qr=qr, kr=kr, dgs=dgs):
                    for h in range(HPC):
                        tr_ps = trps.tile([128, 128], F32, tag="tr")
                        nc.tensor.matmul(tr_ps, lhsT=qr[:, h, :],
                                         rhs=dgs[h], start=True, stop=True)
                        nc.vector.tensor_copy(out=qT_all[:, h, ts], in_=tr_ps)
                    tr_ps = trps.tile([128, 128], F32, tag="tr")
                    nc.tensor.matmul(tr_ps, lhsT=kr, rhs=dgs[HPC],
                                     start=True, stop=True)
                    nc.vector.tensor_copy(out=kT_all[:, ts], in_=tr_ps)
                pend_tr.append(emit_tr)
            flush_tr()

        # -------- Phase 2+3: attention (group-major) fused with WO ---------
        n_groups = SB // 4
        with tc.tile_pool(name="exp", bufs=3) as exp_pool, \
             tc.tile_pool(name="attn", bufs=2) as attn_pool, \
             tc.tile_pool(name="avsb", bufs=2) as avsb_pool, \
             tc.tile_pool(name="rcpp", bufs=2) as rcp_pool, \
             tc.tile_pool(name="s_ps", bufs=2, space="PSUM") as s_ps_pool, \
             tc.tile_pool(name="sm_ps", bufs=1, space="PSUM") as sm_ps_pool, \
             tc.tile_pool(name="av_ps", bufs=2, space="PSUM") as av_ps_pool, \
             tc.tile_pool(name="rbc_ps", bufs=1, space="PSUM") as rbc_pool:

            pend_norm = []        # deferred normalize emissions

            def flush_norm():
                for emit in pend_norm:
                    emit()
                del pend_norm[:]

            for g in range(n_groups):
                rbs = list(range(g * 4, g * 4 + 4))
                eblks = [_ext(rb, p) // 128 for rb in rbs]
                gmax = max(eblks)
                qsl = slice(g * 512, (g + 1) * 512)
                attnT = attn_pool.tile([128, HPC, 512], F32R, tag="attnT")

                for h in range(HPC):
                    sm_ps = sm_ps_pool.tile([1, 512], F32, tag="sm")
                    av_ps = av_ps_pool.tile([128, 512], F32, tag="av")
                    pairs = list(range(0, gmax, 2))
                    exs = {}
                    pend_sum = []

                    def emit_sums(kbp, gmax=gmax, sm_ps=sm_ps, av_ps=av_ps,
                                  exs=exs):
                        ex, npair = exs[kbp]
                        for j in range(npair):
                            kb = kbp + j
                            exj = ex[:, j, :]
                            nc.tensor.matmul(sm_ps, lhsT=ones_col,
                                             rhs=exj, start=(kb == 0),
                                             stop=(kb == gmax - 1))
                            nc.tensor.matmul(av_ps,
                                             lhsT=v_all[:, kb, :],
                                             rhs=exj, start=(kb == 0),
                                             stop=(kb == gmax - 1))

                    for kbp in pairs:
                        npair = min(2, gmax - kbp)
                        s_ps = s_ps_pool.tile([128, 1024], F32, tag="s")
                        for j in range(npair):
                            kb = kbp + j
                            o = j * 512
                            nc.tensor.matmul(
                                s_ps[:, o:o + 512],
                                lhsT=kT_all[:, kb * 128:(kb + 1) * 128],
                                rhs=qT_all[:, h, qsl],
                                start=True, stop=True)
                            # mask q-columns whose extent <= kb (ascending
                            # extents -> always a prefix of the group)
                            jm = sum(1 for e in eblks if e <= kb)
                            if jm > 0:
                                nc.vector.memset(s_ps[:, o:o + jm * 128], NEG)
                            # causal diagonal block (rows >= p)
                            ri_d = kb - g * 4
                            if 0 <= ri_d < 4 and kb * 128 >= p \
                                    and eblks[ri_d] == kb + 1:
                                od = o + ri_d * 128
                                nc.vector.tensor_add(
                                    s_ps[:, od:od + 128],
                                    s_ps[:, od:od + 128], dmask_sb)
                        ex = exp_pool.tile([128, 2, 512], F32R, tag="ex")
                        nc.scalar.activation(
                            out=ex[:, 0:npair, :],
                            in_=s_ps[:, 0:npair * 512],
                            func=AF.Exp, scale=SOFT_SCALE)
                        exs[kbp] = (ex, npair)
                        # sums/AV of the previous pair run after this pair's
                        # scores, overlapping the ACT exp latency.
                        if pend_sum:
                            emit_sums(pend_sum.pop())
                        pend_sum.append(kbp)
                    emit_sums(pend_sum.pop())

                    # reciprocal on DVE; normalization deferred one head so
                    # the PE isn't stalled on it.
                    rcp = rcp_pool.tile([1, 512], F32R, tag="rcp")
                    with nc.allow_low_precision(reason="f32r softmax rcp"):
                        nc.vector.reciprocal(out=rcp, in_=sm_ps)
                    av_sb = avsb_pool.tile([128, 512], F32, tag="av_sb")
                    nc.scalar.copy(out=av_sb, in_=av_ps)

                    def emit_norm(h=h, rcp=rcp, av_sb=av_sb, attnT=attnT):
                        rbc = rbc_pool.tile([128, 512], F32, tag="rbc")
                        nc.tensor.matmul(rbc, lhsT=ones_row, rhs=rcp,
                                         start=True, stop=True)
                        nc.vector.tensor_mul(attnT[:, h, :], av_sb, rbc)
                    flush_norm()
                    pend_norm.append(emit_norm)
                flush_norm()

                # ---- WO for this token chunk (all 4 heads ready) ----
                for db in range(DB):
                    y_ps = av_ps_pool.tile([128, 512], F32, tag="av")
                    for hb in range(HPC):
                        nc.tensor.matmul(
                            y_ps,
                            lhsT=wo_sb[:, hb, db * 128:(db + 1) * 128],
                            rhs=attnT[:, hb, :],
                            start=(hb == 0), stop=(hb == HPC - 1))
                    y_sb = avsb_pool.tile([128, 512], F32, tag="y_sb")
                    nc.vector.tensor_copy(out=y_sb, in_=y_ps)
                    nc.sync.dma_start(
                        out=yT[db * 128:(db + 1) * 128,
                               g * 512:(g + 1) * 512],
                        in_=y_sb)

    if legalize:
        _legalize_waits(nc)
    return nc


def _prep_inputs(x, cos, sin, wq, wk, wv, wo, q_gamma, k_gamma, p):
    """Build the 8 per-core input maps."""
    cos2 = np.asarray(cos, np.float32).reshape(S, HD)
    sin2 = np.asarray(sin, np.float32).reshape(S, HD)
    qg = np.asarray(q_gamma, np.float32)
    kg = np.asarray(k_gamma, np.float32)
    h = HD // 2
    qg_rot = np.concatenate([qg[h:], qg[:h]])
    kg_rot = np.concatenate([kg[h:], kg[:h]])
    cos_q = np.ascontiguousarray(cos2 * qg)
    sin_q = np.ascontiguousarray(sin2 * qg_rot)
    cos_k = np.ascontiguousarray(cos2 * kg)
    sin_k = np.ascontiguousarray(sin2 * kg_rot)
    # fold the rotate-half sign into the low halves of sin
    sin_q[:, :h] *= -1.0
    sin_k[:, :h] *= -1.0

    ii = np.arange(128)
    dmask = np.where(ii[:, None] <= ii[None, :], 0.0, NEG).astype(np.float32)

    x = np.asarray(x, np.float32)
    wq = np.asarray(wq, np.float32)
    wk = np.asarray(wk, np.float32)
    wv = np.asarray(wv, np.float32)
    wo = np.asarray(wo, np.float32)

    xT = [np.ascontiguousarray(x[b].T) for b in range(B)]
    in_maps = []
    for c in range(N_CORES):
        b, g = divmod(c, N_CORES // B)
        h0 = g * HPC
        kv = h0 // (NH // KVH)
        wqTc = np.ascontiguousarray(wq[h0 * HD:(h0 + HPC) * HD, :].T)
        wkvTc = np.ascontiguousarray(
            np.concatenate([wk[kv * HD:(kv + 1) * HD, :],
                            wv[kv * HD:(kv + 1) * HD, :]], axis=0).T)
        woTc = np.ascontiguousarray(wo[:, h0 * HD:(h0 + HPC) * HD].T)
        in_maps.append({
            "xT": xT[b], "wqT": wqTc, "wkvT": wkvTc, "woT": woTc,
            "cos_q": cos_q, "sin_q": sin_q, "cos_k": cos_k, "sin_k": sin_k,
            "dmask": dmask,
        })
    return in_maps


def _gather(results):
    y = np.zeros((B, S, D), dtype=np.float32)
    for c in range(N_CORES):
        b = c // (N_CORES // B)
        y[b] += results[c]["yT"].T
    return y


def kernel(x, cos, sin, wq, wk, wv, wo, q_gamma, k_gamma, signal_token_num):
    p = int(signal_token_num)
    assert p % 128 == 0 and 0 <= p <= S, f"unsupported signal_token_num {p}"

    nc = build_core_kernel(p)
    in_maps = _prep_inputs(x, cos, sin, wq, wk, wv, wo, q_gamma, k_gamma, p)
    res = run_bass_kernel_spmd(nc, in_maps, list(range(N_CORES)))
    return _gather(res.results)


def _install_ntff_hook():
    """The container's antenv lacks axon_hooks; replicate the boot-time NTFF
    profile hook (ctypes into libaxon_pjrt.so) and register the module."""
    import sys
    import types
    import ctypes
    import contextlib

    if "antenv.axon_hooks" in sys.modules:
        return
    so_path = "/opt/axon/libaxon_pjrt.so"
    lib = ctypes.CDLL(so_path)
    if not hasattr(lib, "axon_start_nrt_profile"):
        return
    lib.axon_start_nrt_profile.argtypes = [
        ctypes.POINTER(ctypes.c_int64), ctypes.c_size_t]
    lib.axon_start_nrt_profile.restype = ctypes.c_int64
    lib.axon_stop_nrt_profile.argtypes = [ctypes.c_char_p]
    lib.axon_stop_nrt_profile.restype = ctypes.c_int64

    @contextlib.contextmanager
    def _hook(output_dir, device_ids):
        import jax
        jax.devices()
        if device_ids:
            ids = (ctypes.c_int64 * len(device_ids))(*device_ids)
            rc = lib.axon_start_nrt_profile(ids, len(device_ids))
        else:
            rc = lib.axon_start_nrt_profile(None, 0)
        if rc != 0:
            raise RuntimeError(f"axon_start_nrt_profile rc={rc}")
        try:
            yield
        finally:
            n = lib.axon_stop_nrt_profile(str(output_dir).encode())
            print(f"profile: {n} file(s) written to {output_dir}")

    import antenv
    mod = types.ModuleType("antenv.axon_hooks")
    mod.get_axon_ntff_profile_hook = lambda: _hook
    mod.set_axon_ntff_profile_hook = lambda h: None
    sys.modules["antenv.axon_hooks"] = mod
    antenv.axon_hooks = mod


def profile_once(inputs):
    """Run once with NTFF tracing; return max per-core exec time in ns."""
    import concourse.bass_utils as bu
    bu.upload_artifacts = lambda tmpdir: ""   # no bucket access here
    _install_ntff_hook()
    p = int(inputs["signal_token_num"])
    nc = build_core_kernel(p)
    in_maps = _prep_inputs(
        inputs["x"], inputs["cos"], inputs["sin"], inputs["wq"], inputs["wk"],
        inputs["wv"], inputs["wo"], inputs["q_gamma"], inputs["k_gamma"], p)
    try:
        res = bu.run_bass_kernel_spmd(nc, in_maps, list(range(N_CORES)),
                                      trace=True,
                                      trace_cores=list(range(N_CORES)))
        return res.exec_time_ns
    except Exception as e:
        print(f"profile failed: {type(e).__name__}: {e}")
        return None


# revision 13
# speedup vs baseline: 1.4023x; 1.4023x over previous
"""Trainium2 Bass kernel for GQA attention with QK-RMSNorm, RoPE and a
bidirectional-prefix + causal mask (sparse_attention problem).

Reference computation (fp32):
  xq = x @ wq.T; xk = x @ wk.T; xv = x @ wv.T   (per-head RMSNorm on q,k)
  rope(q), rope(k); repeat kv heads 8x
  scores = q k^T / sqrt(128); mask = causal OR (i<p & j<p)
  out = softmax(scores) @ v;  y = out @ wo.T

Sharding: 8 cores = 2 batches x 4 head-groups (4 query heads each, sharing
one KV head).  Each core computes a partial y^T (its 4 heads' contribution);
the host sums the 4 partials per batch and transposes back.

v2 design notes (vs the first working version):
  * No fp32->fp32r staging copies: matmul-consumed DRAM tensors and SBUF
    tiles are declared float32r (same 32-bit layout host-side) so DMA
    lands them directly; engine-written operands write f32r natively.
  * RMSNorm sum-of-squares runs on the Scalar engine (Square + accum_out),
    the per-token 1/sqrt scale is applied FOR FREE by the PE transpose:
    instead of transposing with the identity, we transpose with
    diag(rq) so qT = q.T @ diag(rq) lands pre-scaled.
  * RoPE sign is folded into sin (host negates the low half), so rope is
    3 wide DVE multiplies + 1 add across all 4 heads at once, reading the
    projection results directly from PSUM.
  * K-path rope + the diag(rq) builds run on the otherwise-idle GPSIMD.
  * Softmax: exp on ACT (per 2-block pair), row sums via a ones-matmul,
    reciprocal on DVE, and the [1,512] -> [128,512] broadcast is a rank-1
    PE matmul (ones outer product) instead of a DRAM round-trip.
  * Single PE instruction stream ordered so the PE never has a long gap
    (projection MMs -> previous tb's transposes -> ... -> attention),
    keeping the HAM clock-gate at 8/8.
  * ACT program order is strictly {Square,Copy,Sqrt} then {Exp,Copy}, so
    exactly two activation-table loads happen.

TRN2 ISA allows ONE sync-wait per instruction and walrus does not split
multi-wait instructions, so `_legalize_waits` rewrites the emitted BIR,
moving excess waits onto preceding same-engine NoOps.
"""
import math
import numpy as np
from contextlib import ExitStack

import bass_rust
import concourse.bass as bass
import concourse.mybir as mybir
import concourse.tile as tile
from concourse.bass_utils import run_bass_kernel_spmd
from concourse.masks import make_identity

F32 = mybir.dt.float32
F32R = mybir.dt.float32r
AF = mybir.ActivationFunctionType

B, S, D = 2, 2048, 2048
NH, KVH, HD = 16, 2, 128
HPC = 4                      # query heads per core
N_CORES = 8
EPS = 1e-6
SOFT_SCALE = 1.0 / math.sqrt(HD)
NEG = -1.0e30

SB = S // 128                # 16 token blocks
DB = D // 128                # 16 contraction blocks

_lgw_counter = [0]


def _legalize_waits(nc, cap=1):
    """Move all-but-`cap` sync waits of every instruction onto preceding
    same-engine NoOps (TRN2 EVENTS block has a single wait slot)."""
    for fn in nc.m.functions:
        for blk in fn.blocks:
            out = []
            changed = False
            for inst in blk.instructions:
                si = inst.sync_info
                waits = list(si.on_wait) if si is not None and si.on_wait else []
                if len(waits) > cap:
                    changed = True
                    move, keep = waits[:-cap], waits[-cap:]
                    for w in move:
                        n = bass_rust.InstNoOp(name=f"LGW-{_lgw_counter[0]}")
                        _lgw_counter[0] += 1
                        n.engine = inst.engine
                        n.sync_info = mybir.SyncInfo(on_wait=[w], on_update=[])
                        out.append(n)
                    inst.sync_info = mybir.SyncInfo(
                        on_wait=keep, on_update=list(si.on_update or []))
                out.append(inst)
            if changed:
                blk.instructions = out
    return nc


def _ext(rb, p):
    """Key extent attended by query row-block rb (rows rb*128 .. rb*128+127)."""
    lo, hi = rb * 128, (rb + 1) * 128
    if hi <= p:
        return p              # prefix rows attend the full prefix [0, p)
    return hi                 # causal rows attend [0, hi), diag-masked


def build_core_kernel(p, legalize=True):
    """One SPMD program; per-core behavior differs only via input data."""
    nc = bass.Bass()

    xT = nc.dram_tensor("xT", [D, S], F32R, kind="ExternalInput")
    wqT = nc.dram_tensor("wqT", [D, HPC * HD], F32R, kind="ExternalInput")
    wkvT = nc.dram_tensor("wkvT", [D, 2 * HD], F32R, kind="ExternalInput")
    woT = nc.dram_tensor("woT", [HPC * HD, D], F32R, kind="ExternalInput")
    cos_q = nc.dram_tensor("cos_q", [S, HD], F32, kind="ExternalInput")
    sin_q = nc.dram_tensor("sin_q", [S, HD], F32, kind="ExternalInput")
    cos_k = nc.dram_tensor("cos_k", [S, HD], F32, kind="ExternalInput")
    sin_k = nc.dram_tensor("sin_k", [S, HD], F32, kind="ExternalInput")
    dmask = nc.dram_tensor("dmask", [128, 128], F32, kind="ExternalInput")
    rcp_scr = nc.dram_tensor("rcp_scr", [SB * HPC, 512], F32)
    yT = nc.dram_tensor("yT", [D, S], F32, kind="ExternalOutput")

    with tile.TileContext(nc) as tc, ExitStack() as octx:
        const = octx.enter_context(tc.tile_pool(name="const", bufs=1))
        ident = const.tile([128, 128], F32)
        make_identity(nc, ident)
        dmask_sb = const.tile([128, 128], F32)
        nc.sync.dma_start(out=dmask_sb, in_=dmask[:, :])
        eps_t = const.tile([128, 1], F32)
        nc.vector.memset(eps_t, EPS)
        ones_f = const.tile([128, 1], F32)
        nc.vector.memset(ones_f, 1.0)
        ones_col = const.tile([128, 1], F32R)
        nc.vector.tensor_copy(out=ones_col, in_=ones_f)

        qkv = octx.enter_context(tc.tile_pool(name="qkv", bufs=1))
        qT_all = qkv.tile([128, HPC, S], F32R)        # [hd, h, tok]
        kT_all = qkv.tile([128, S], F32R)             # [hd, tok]
        v_all = qkv.tile([128, SB, HD], F32R)         # [tok(P), tb, hd]

        wpool = octx.enter_context(tc.tile_pool(name="w", bufs=1))
        wq_sb = wpool.tile([128, DB, HPC * HD], F32R)
        wkv_sb = wpool.tile([128, DB, 2 * HD], F32R)
        wo_sb = wpool.tile([128, HPC, D], F32R)

        # weight DMAs, split across two queues so x(tb0) isn't starved:
        # wq alternates sync/scalar; wkv likewise (behind wq); wo on scalar.
        for kb in range(DB):
            eng = nc.sync if kb % 2 == 0 else nc.scalar
            eng.dma_start(out=wq_sb[:, kb, :],
                          in_=wqT[kb * 128:(kb + 1) * 128, :])

        # ---------------- Phase 1: QKV projections + norm/rope -------------
        with tc.tile_pool(name="xp", bufs=3) as xp, \
             tc.tile_pool(name="cs", bufs=2) as cs, \
             tc.tile_pool(name="rp", bufs=2) as rp, \
             tc.tile_pool(name="dg", bufs=2) as dgp, \
             tc.tile_pool(name="qps", bufs=2, space="PSUM") as qps_pool, \
             tc.tile_pool(name="kvps", bufs=2, space="PSUM") as kvps_pool, \
             tc.tile_pool(name="trps", bufs=4, space="PSUM") as trps:

            pend_tr = []          # deferred transpose emissions

            def flush_tr():
                for emit in pend_tr:
                    emit()
                del pend_tr[:]

            for tb in range(SB):
                ts = slice(tb * 128, (tb + 1) * 128)

                x_h = []
                for half in range(2):
                    xh = xp.tile([128, 8, 128], F32R, tag=f"x{half}")
                    nc.sync.dma_start(
                        out=xh,
                        in_=xT[half * 1024:(half + 1) * 1024, ts]
                        .rearrange("(kb pp) t -> pp kb t", pp=128))
                    x_h.append(xh)
                cq = cs.tile([128, HD], F32, tag="cq")
                nc.sync.dma_start(out=cq, in_=cos_q[ts, :])
                sq = cs.tile([128, HD], F32, tag="sq")
                nc.sync.dma_start(out=sq, in_=sin_q[ts, :])
                ck = cs.tile([128, HD], F32, tag="ck")
                nc.sync.dma_start(out=ck, in_=cos_k[ts, :])
                sk = cs.tile([128, HD], F32, tag="sk")
                nc.sync.dma_start(out=sk, in_=sin_k[ts, :])
                if tb == 0:
                    for kb in range(DB):
                        eng = nc.sync if kb % 2 == 0 else nc.scalar
                        eng.dma_start(out=wkv_sb[:, kb, :],
                                      in_=wkvT[kb * 128:(kb + 1) * 128, :])
                if tb == 1:
                    for hb in range(HPC):
                        nc.scalar.dma_start(
                            out=wo_sb[:, hb, :],
                            in_=woT[hb * 128:(hb + 1) * 128, :])

                q_ps = qps_pool.tile([128, HPC * HD], F32, tag="q")
                kv_ps = kvps_pool.tile([128, 2 * HD], F32, tag="kv")
                for kb in range(DB):
                    xb = x_h[kb // 8][:, kb % 8, :]
                    nc.tensor.matmul(q_ps, lhsT=xb, rhs=wq_sb[:, kb, :],
                                     start=(kb == 0), stop=(kb == DB - 1))
                    nc.tensor.matmul(kv_ps, lhsT=xb, rhs=wkv_sb[:, kb, :],
                                     start=(kb == 0), stop=(kb == DB - 1))
                # previous tb's transposes keep the PE busy while this tb's
                # rope/rms runs on DVE/ACT/GPSIMD.
                flush_tr()

                # --- RMS stats on ACT (Square accumulates sum along free) ---
                ms = rp.tile([128, 8], F32, tag="ms")
                scr = rp.tile([128, HD], F32, tag="scr")
                for h in range(HPC):
                    nc.scalar.activation(
                        out=scr, in_=q_ps[:, h * HD:(h + 1) * HD],
                        func=AF.Square, accum_out=ms[:, h:h + 1])
                k_sb = rp.tile([128, HD], F32, tag="k_sb")
                nc.scalar.copy(out=k_sb, in_=kv_ps[:, 0:HD])
                nc.scalar.copy(out=v_all[:, tb, :], in_=kv_ps[:, HD:])
                nc.scalar.activation(out=scr, in_=k_sb, func=AF.Square,
                                     accum_out=ms[:, HPC:HPC + 1])
                srq = rp.tile([128, 8], F32, tag="srq")
                nc.scalar.activation(out=srq[:, 0:HPC + 1],
                                     in_=ms[:, 0:HPC + 1], func=AF.Sqrt,
                                     bias=eps_t, scale=1.0 / HD)
                rq = rp.tile([128, 8], F32, tag="rq")
                nc.vector.reciprocal(out=rq[:, 0:HPC + 1],
                                     in_=srq[:, 0:HPC + 1])

                # --- rope q: wide over all 4 heads, straight from PSUM ---
                # cos/sin broadcast across heads via stride-0 views
                h2 = HD // 2
                cq4 = bass.AP(tensor=cq.tensor, offset=cq.offset,
                              ap=[list(cq.ap[0]), [0, HPC]] + list(cq.ap[1:]))
                sq4 = bass.AP(tensor=sq.tensor, offset=sq.offset,
                              ap=[list(sq.ap[0]), [0, HPC]] + list(sq.ap[1:]))
                q4 = q_ps[:, :].rearrange("pp (h d) -> pp h d", h=HPC)
                t14 = rp.tile([128, HPC, HD], F32, tag="t14")
                nc.vector.tensor_mul(t14, q4, cq4)
                t2p4 = rp.tile([128, HPC, HD], F32, tag="t2p4")
                # low half: q_hi * (-sin_lo)  (sign folded on host)
                nc.vector.tensor_mul(
                    t2p4[:, :, 0:h2], q4[:, :, h2:HD], sq4[:, :, 0:h2])
                nc.vector.tensor_mul(
                    t2p4[:, :, h2:HD], q4[:, :, 0:h2], sq4[:, :, h2:HD])
                qr = rp.tile([128, HPC, HD], F32R, tag="qr")
                nc.vector.tensor_add(qr, t14, t2p4)

                # --- rope k on GPSIMD (SBUF-only engine) ---
                kr = rp.tile([128, HD], F32R, tag="kr")
                t1k = rp.tile([128, HD], F32, tag="t1k")
                nc.gpsimd.tensor_mul(t1k, k_sb, ck)
                t2k = rp.tile([128, HD], F32, tag="t2k")
                nc.gpsimd.tensor_mul(t2k[:, 0:h2], k_sb[:, h2:HD], sk[:, 0:h2])
                nc.gpsimd.tensor_mul(t2k[:, h2:HD], k_sb[:, 0:h2], sk[:, h2:HD])
                nc.gpsimd.tensor_add(kr, t1k, t2k)

                # --- diag(rq) tiles on GPSIMD ---
                dgs = []
                for j in range(HPC + 1):
                    dg = dgp.tile([128, 128], F32R, tag=f"dg{j}")
                    nc.vector.tensor_scalar_mul(dg, ident, rq[:, j:j + 1])
                    dgs.append(dg)

                # --- transposes (deferred into next tb's PE slot):
                #     qT = qr.T @ diag(rq) applies the RMS scale for free ---
                def emit_tr(tb=tb, ts=ts, qr=qr, kr=kr, dgs=dgs):
                    for h in range(HPC):
                        tr_ps = trps.tile([128, 128], F32, tag="tr")
                        nc.tensor.matmul(tr_ps, lhsT=qr[:, h, :],
                                         rhs=dgs[h], start=True, stop=True)
                        if h >= 2:
                            nc.scalar.copy(out=qT_all[:, h, ts], in_=tr_ps)
                        else:
                            nc.vector.tensor_copy(out=qT_all[:, h, ts],
                                                  in_=tr_ps)
                    tr_ps = trps.tile([128, 128], F32, tag="tr")
                    nc.tensor.matmul(tr_ps, lhsT=kr, rhs=dgs[HPC],
                                     start=True, stop=True)
                    nc.vector.tensor_copy(out=kT_all[:, ts], in_=tr_ps)
                pend_tr.append(emit_tr)
            flush_tr()

        # -------- Phase 2+3: attention (group-major) fused with WO ---------
        # Deferred-emission software pipeline, one global pair-iteration
        # counter.  Per pair: scores MMs -> ACT exp -> (post-exp masking on
        # the SBUF exp tile, so no DVE op ever gates the exp) ; sums/AV
        # matmuls drain TWO pair-iterations later so the PE never waits on
        # the ACT exp round-trip.  Softmax normalization runs entirely on
        # ACT + DMA: 1/Z = exp(-ln Z) with a DRAM round-trip broadcast
        # (Ln and Exp share one activation table set).  WO for group g is
        # emitted after head (g+1, 0), giving the last head's
        # normalization a full head of slack; its y tiles go out via DMA
        # straight from PSUM.  PSUM: scores/y 3x2 banks + sums 1 + AV 1.
        n_groups = SB // 4
        with tc.tile_pool(name="exp", bufs=3) as exp_pool, \
             tc.tile_pool(name="attn", bufs=2) as attn_pool, \
             tc.tile_pool(name="avsb", bufs=2) as avsb_pool, \
             tc.tile_pool(name="rcpp", bufs=2) as rcp_pool, \
             tc.tile_pool(name="s_ps", bufs=3, space="PSUM") as s_ps_pool, \
             tc.tile_pool(name="sm_ps", bufs=1, space="PSUM") as sm_ps_pool, \
             tc.tile_pool(name="av_ps", bufs=1, space="PSUM") as av_ps_pool:

            it = [0]              # global pair-iteration counter
            sum_q = []            # (emit_fn,) sums/AV, drained at depth 2
            fin_q = []            # (ready_iter, emit_fn) deferred finishers
            wo_q = []             # deferred WO group emissions

            def drain(keep_sums=2):
                # sums first: a finisher reads av/sm tiles, so the deferred
                # matmuls that write them must be emitted before it
                while len(sum_q) > keep_sums:
                    sum_q.pop(0)()
                while fin_q and fin_q[0][0] <= it[0]:
                    fin_q.pop(0)[1]()

            for g in range(n_groups):
                rbs = list(range(g * 4, g * 4 + 4))
                eblks = [_ext(rb, p) // 128 for rb in rbs]
                gmax = max(eblks)
                qsl = slice(g * 512, (g + 1) * 512)
                attnT = attn_pool.tile([128, HPC, 512], F32R, tag="attnT")

                for h in range(HPC):
                    if h == 1 and wo_q:
                        wo_q.pop(0)()     # WO(g-1): its attnT is long ready
                    sm_ps = sm_ps_pool.tile([1, 512], F32, tag="sm")
                    av_ps = av_ps_pool.tile([128, 512], F32, tag="av")

                    for kbp in range(0, gmax, 2):
                        npair = min(2, gmax - kbp)
                        s_ps = s_ps_pool.tile([128, 1024], F32, tag="s")
                        for j in range(npair):
                            kb = kbp + j
                            nc.tensor.matmul(
                                s_ps[:, j * 512:j * 512 + 512],
                                lhsT=kT_all[:, kb * 128:(kb + 1) * 128],
                                rhs=qT_all[:, h, qsl],
                                start=True, stop=True)
                        ex = exp_pool.tile([128, 2, 512], F32R, tag="ex")
                        nc.scalar.activation(
                            out=ex[:, 0:npair, :],
                            in_=s_ps[:, 0:npair * 512],
                            func=AF.Exp, scale=SOFT_SCALE)
                        # post-exp masking on SBUF (never gates the exp):
                        # zero q-columns whose extent <= kb (a prefix of the
                        # group) and multiply the causal diagonal block by
                        # the 0/1 lower-triangle mask.
                        for j in range(npair):
                            kb = kbp + j
                            jm = sum(1 for e in eblks if e <= kb)
                            if jm > 0:
                                # memset can't write f32r; scale by 0 instead
                                nc.vector.tensor_scalar_mul(
                                    ex[:, j, 0:jm * 128],
                                    ex[:, j, 0:jm * 128], 0.0)
                            ri_d = kb - g * 4
                            if 0 <= ri_d < 4 and kb * 128 >= p \
                                    and eblks[ri_d] == kb + 1:
                                od = ri_d * 128
                                nc.vector.tensor_mul(
                                    ex[:, j, od:od + 128],
                                    ex[:, j, od:od + 128], dmask_sb)

                        def emit_sums(ex=ex, npair=npair, kbp=kbp,
                                      gmax=gmax, sm_ps=sm_ps, av_ps=av_ps):
                            for j in range(npair):
                                kb = kbp + j
                                exj = ex[:, j, :]
                                nc.tensor.matmul(sm_ps, lhsT=ones_col,
                                                 rhs=exj, start=(kb == 0),
                                                 stop=(kb == gmax - 1))
                                nc.tensor.matmul(av_ps,
                                                 lhsT=v_all[:, kb, :],
                                                 rhs=exj, start=(kb == 0),
                                                 stop=(kb == gmax - 1))
                        sum_q.append(emit_sums)
                        it[0] += 1
                        drain(keep_sums=2)

                    # head finishers, deferred 2 pair-iterations:
                    #   stage A: av -> SBUF (ACT), lnZ (ACT), lnZ -> DRAM
                    #   stage B (2 more iters): bcast read, 1/Z = exp(-lnZ)
                    #            (ACT), attnT = av * 1/Z (DVE)
                    slot = g * HPC + h
                    def fin_a(slot=slot, sm_ps=sm_ps, av_ps=av_ps,
                              h=h, attnT=attnT):
                        lnz = rcp_pool.tile([1, 512], F32, tag="lnz")
                        nc.scalar.activation(out=lnz, in_=sm_ps, func=AF.Ln)
                        nc.sync.dma_start(out=rcp_scr[slot:slot + 1, :],
                                          in_=lnz)
                        av_sb = avsb_pool.tile([128, 512], F32, tag="av_sb")
                        nc.scalar.copy(out=av_sb, in_=av_ps)
                        def fin_b(slot=slot, av_sb=av_sb, h=h, attnT=attnT):
                            lbc = rcp_pool.tile([128, 512], F32, tag="lbc")
                            drap = rcp_scr[slot:slot + 1, :]
                            bcast = bass.AP(tensor=drap.tensor,
                                            offset=drap.offset,
                                            ap=[[0, 128]] + list(drap.ap[1:]))
                            nc.sync.dma_start(out=lbc, in_=bcast)
                            rbc = rcp_pool.tile([128, 512], F32, tag="rbc")
                            nc.scalar.activation(out=rbc, in_=lbc,
                                                 func=AF.Exp, scale=-1.0)
                            nc.vector.tensor_mul(attnT[:, h, :], av_sb, rbc)
                        fin_q.append((it[0] + 2, fin_b))
                    fin_q.append((it[0] + 2, fin_a))

                # ---- WO for this token chunk, deferred one head ----
                def emit_wo(g=g, attnT=attnT):
                    for db in range(DB):
                        y_ps = s_ps_pool.tile([128, 1024], F32, tag="s")
                        for hb in range(HPC):
                            nc.tensor.matmul(
                                y_ps[:, 0:512],
                                lhsT=wo_sb[:, hb, db * 128:(db + 1) * 128],
                                rhs=attnT[:, hb, :],
                                start=(hb == 0), stop=(hb == HPC - 1))
                        y_sb = avsb_pool.tile([128, 512], F32, tag="y_sb")
                        nc.vector.tensor_copy(out=y_sb, in_=y_ps[:, 0:512])
                        nc.sync.dma_start(
                            out=yT[db * 128:(db + 1) * 128,
                                   g * 512:(g + 1) * 512],
                            in_=y_sb)
                wo_q.append(emit_wo)

            while sum_q or fin_q:
                it[0] += 1
                drain(keep_sums=0)
            while wo_q:
                wo_q.pop(0)()

    if legalize:
        _legalize_waits(nc)
    return nc


def _prep_inputs(x, cos, sin, wq, wk, wv, wo, q_gamma, k_gamma, p):
    """Build the 8 per-core input maps."""
    cos2 = np.asarray(cos, np.float32).reshape(S, HD)
    sin2 = np.asarray(sin, np.float32).reshape(S, HD)
    qg = np.asarray(q_gamma, np.float32)
    kg = np.asarray(k_gamma, np.float32)
    h = HD // 2
    qg_rot = np.concatenate([qg[h:], qg[:h]])
    kg_rot = np.concatenate([kg[h:], kg[:h]])
    cos_q = np.ascontiguousarray(cos2 * qg)
    sin_q = np.ascontiguousarray(sin2 * qg_rot)
    cos_k = np.ascontiguousarray(cos2 * kg)
    sin_k = np.ascontiguousarray(sin2 * kg_rot)
    # fold the rotate-half sign into the low halves of sin
    sin_q[:, :h] *= -1.0
    sin_k[:, :h] *= -1.0

    ii = np.arange(128)
    dmask = (ii[:, None] <= ii[None, :]).astype(np.float32)

    x = np.asarray(x, np.float32)
    wq = np.asarray(wq, np.float32)
    wk = np.asarray(wk, np.float32)
    wv = np.asarray(wv, np.float32)
    wo = np.asarray(wo, np.float32)

    xT = [np.ascontiguousarray(x[b].T) for b in range(B)]
    in_maps = []
    for c in range(N_CORES):
        b, g = divmod(c, N_CORES // B)
        h0 = g * HPC
        kv = h0 // (NH // KVH)
        wqTc = np.ascontiguousarray(wq[h0 * HD:(h0 + HPC) * HD, :].T)
        wkvTc = np.ascontiguousarray(
            np.concatenate([wk[kv * HD:(kv + 1) * HD, :],
                            wv[kv * HD:(kv + 1) * HD, :]], axis=0).T)
        woTc = np.ascontiguousarray(wo[:, h0 * HD:(h0 + HPC) * HD].T)
        in_maps.append({
            "xT": xT[b], "wqT": wqTc, "wkvT": wkvTc, "woT": woTc,
            "cos_q": cos_q, "sin_q": sin_q, "cos_k": cos_k, "sin_k": sin_k,
            "dmask": dmask,
        })
    return in_maps


def _gather(results):
    y = np.zeros((B, S, D), dtype=np.float32)
    for c in range(N_CORES):
        b = c // (N_CORES // B)
        y[b] += results[c]["yT"].T
    return y


def kernel(x, cos, sin, wq, wk, wv, wo, q_gamma, k_gamma, signal_token_num):
    p = int(signal_token_num)
    assert p % 128 == 0 and 0 <= p <= S, f"unsupported signal_token_num {p}"

    nc = build_core_kernel(p)
    in_maps = _prep_inputs(x, cos, sin, wq, wk, wv, wo, q_gamma, k_gamma, p)
    res = run_bass_kernel_spmd(nc, in_maps, list(range(N_CORES)))
    return _gather(res.results)


def _install_ntff_hook():
    """The container's antenv lacks axon_hooks; replicate the boot-time NTFF
    profile hook (ctypes into libaxon_pjrt.so) and register the module."""
    import sys
    import types
    import ctypes
    import contextlib

    if "antenv.axon_hooks" in sys.modules:
        return
    so_path = "/opt/axon/libaxon_pjrt.so"
    lib = ctypes.CDLL(so_path)
    if not hasattr(lib, "axon_start_nrt_profile"):
        return
    lib.axon_start_nrt_profile.argtypes = [
        ctypes.POINTER(ctypes.c_int64), ctypes.c_size_t]
    lib.axon_start_nrt_profile.restype = ctypes.c_int64
    lib.axon_stop_nrt_profile.argtypes = [ctypes.c_char_p]
    lib.axon_stop_nrt_profile.restype = ctypes.c_int64

    @contextlib.contextmanager
    def _hook(output_dir, device_ids):
        import jax
        jax.devices()
        if device_ids:
            ids = (ctypes.c_int64 * len(device_ids))(*device_ids)
            rc = lib.axon_start_nrt_profile(ids, len(device_ids))
        else:
            rc = lib.axon_start_nrt_profile(None, 0)
        if rc != 0:
            raise RuntimeError(f"axon_start_nrt_profile rc={rc}")
        try:
            yield
        finally:
            n = lib.axon_stop_nrt_profile(str(output_dir).encode())
            print(f"profile: {n} file(s) written to {output_dir}")

    import antenv
    mod = types.ModuleType("antenv.axon_hooks")
    mod.get_axon_ntff_profile_hook = lambda: _hook
    mod.set_axon_ntff_profile_hook = lambda h: None
    sys.modules["antenv.axon_hooks"] = mod
    antenv.axon_hooks = mod


def profile_once(inputs):
    """Run once with NTFF tracing; return max per-core exec time in ns."""
    import concourse.bass_utils as bu
    bu.upload_artifacts = lambda tmpdir: ""   # no bucket access here
    _install_ntff_hook()
    p = int(inputs["signal_token_num"])
    nc = build_core_kernel(p)
    in_maps = _prep_inputs(
        inputs["x"], inputs["cos"], inputs["sin"], inputs["wq"], inputs["wk"],
        inputs["wv"], inputs["wo"], inputs["q_gamma"], inputs["k_gamma"], p)
    try:
        res = bu.run_bass_kernel_spmd(nc, in_maps, list(range(N_CORES)),
                                      trace=True,
                                      trace_cores=list(range(N_CORES)))
        return res.exec_time_ns
    except Exception as e:
        print(f"profile failed: {type(e).__name__}: {e}")
        return None


# revision 15
# speedup vs baseline: 1.5660x; 1.1167x over previous
"""Trainium2 Bass kernel for GQA attention with QK-RMSNorm, RoPE and a
bidirectional-prefix + causal mask (sparse_attention problem).

Reference computation (fp32):
  xq = x @ wq.T; xk = x @ wk.T; xv = x @ wv.T   (per-head RMSNorm on q,k)
  rope(q), rope(k); repeat kv heads 8x
  scores = q k^T / sqrt(128); mask = causal OR (i<p & j<p)
  out = softmax(scores) @ v;  y = out @ wo.T

Sharding: 8 cores = 2 batches x 4 head-groups (4 query heads each, sharing
one KV head).  Each core computes a partial y^T (its 4 heads' contribution);
the host sums the 4 partials per batch and transposes back.

v2 design notes (vs the first working version):
  * No fp32->fp32r staging copies: matmul-consumed DRAM tensors and SBUF
    tiles are declared float32r (same 32-bit layout host-side) so DMA
    lands them directly; engine-written operands write f32r natively.
  * RMSNorm sum-of-squares runs on the Scalar engine (Square + accum_out),
    the per-token 1/sqrt scale is applied FOR FREE by the PE transpose:
    instead of transposing with the identity, we transpose with
    diag(rq) so qT = q.T @ diag(rq) lands pre-scaled.
  * RoPE sign is folded into sin (host negates the low half), so rope is
    3 wide DVE multiplies + 1 add across all 4 heads at once, reading the
    projection results directly from PSUM.
  * K-path rope + the diag(rq) builds run on the otherwise-idle GPSIMD.
  * Softmax: exp on ACT (per 2-block pair), row sums via a ones-matmul,
    reciprocal on DVE, and the [1,512] -> [128,512] broadcast is a rank-1
    PE matmul (ones outer product) instead of a DRAM round-trip.
  * Single PE instruction stream ordered so the PE never has a long gap
    (projection MMs -> previous tb's transposes -> ... -> attention),
    keeping the HAM clock-gate at 8/8.
  * ACT program order is strictly {Square,Copy,Sqrt} then {Exp,Copy}, so
    exactly two activation-table loads happen.

TRN2 ISA allows ONE sync-wait per instruction and walrus does not split
multi-wait instructions, so `_legalize_waits` rewrites the emitted BIR,
moving excess waits onto preceding same-engine NoOps.
"""
import math
import numpy as np
from contextlib import ExitStack

import bass_rust
import concourse.bass as bass
import concourse.mybir as mybir
import concourse.tile as tile
from concourse.bass_utils import run_bass_kernel_spmd
from concourse.masks import make_identity

F32 = mybir.dt.float32
F32R = mybir.dt.float32r
AF = mybir.ActivationFunctionType

B, S, D = 2, 2048, 2048
NH, KVH, HD = 16, 2, 128
HPC = 4                      # query heads per core
N_CORES = 8
EPS = 1e-6
SOFT_SCALE = 1.0 / math.sqrt(HD)
NEG = -1.0e30

SB = S // 128                # 16 token blocks
DB = D // 128                # 16 contraction blocks

_lgw_counter = [0]


def _legalize_waits(nc, cap=1):
    """Move all-but-`cap` sync waits of every instruction onto preceding
    same-engine NoOps (TRN2 EVENTS block has a single wait slot)."""
    for fn in nc.m.functions:
        for blk in fn.blocks:
            out = []
            changed = False
            for inst in blk.instructions:
                si = inst.sync_info
                waits = list(si.on_wait) if si is not None and si.on_wait else []
                if len(waits) > cap:
                    changed = True
                    move, keep = waits[:-cap], waits[-cap:]
                    for w in move:
                        n = bass_rust.InstNoOp(name=f"LGW-{_lgw_counter[0]}")
                        _lgw_counter[0] += 1
                        n.engine = inst.engine
                        n.sync_info = mybir.SyncInfo(on_wait=[w], on_update=[])
                        out.append(n)
                    inst.sync_info = mybir.SyncInfo(
                        on_wait=keep, on_update=list(si.on_update or []))
                out.append(inst)
            if changed:
                blk.instructions = out
    return nc


def _ext(rb, p):
    """Key extent attended by query row-block rb (rows rb*128 .. rb*128+127)."""
    lo, hi = rb * 128, (rb + 1) * 128
    if hi <= p:
        return p              # prefix rows attend the full prefix [0, p)
    return hi                 # causal rows attend [0, hi), diag-masked


def build_core_kernel(p, legalize=True):
    """One SPMD program; per-core behavior differs only via input data."""
    nc = bass.Bass()

    xP = nc.dram_tensor("xP", [SB, 128, DB, 128], F32R, kind="ExternalInput")
    wqT = nc.dram_tensor("wqT", [D, HPC * HD], F32R, kind="ExternalInput")
    wkvT = nc.dram_tensor("wkvT", [D, 2 * HD], F32R, kind="ExternalInput")
    woT = nc.dram_tensor("woT", [HPC * HD, D], F32R, kind="ExternalInput")
    cs4 = nc.dram_tensor("cs4", [S, 4, HD], F32, kind="ExternalInput")
    dmask = nc.dram_tensor("dmask", [128, 128], F32, kind="ExternalInput")
    rcp_scr = nc.dram_tensor("rcp_scr", [SB * HPC, 512], F32)
    yT = nc.dram_tensor("yT", [D, S], F32, kind="ExternalOutput")

    with tile.TileContext(nc) as tc, ExitStack() as octx:
        const = octx.enter_context(tc.tile_pool(name="const", bufs=1))
        ident = const.tile([128, 128], F32)
        make_identity(nc, ident)
        dmask_sb = const.tile([128, 128], F32)
        nc.sync.dma_start(out=dmask_sb, in_=dmask[:, :])
        eps_t = const.tile([128, 1], F32)
        nc.vector.memset(eps_t, EPS)
        ones_f = const.tile([128, 1], F32)
        nc.vector.memset(ones_f, 1.0)
        ones_col = const.tile([128, 1], F32R)
        nc.vector.tensor_copy(out=ones_col, in_=ones_f)

        qkv = octx.enter_context(tc.tile_pool(name="qkv", bufs=1))
        qT_all = qkv.tile([128, HPC, S], F32R)        # [hd, h, tok]
        kT_all = qkv.tile([128, S], F32R)             # [hd, tok]
        v_all = qkv.tile([128, SB, HD], F32R)         # [tok(P), tb, hd]

        wpool = octx.enter_context(tc.tile_pool(name="w", bufs=1))
        wq_sb = wpool.tile([128, DB, HPC * HD], F32R)
        wkv_sb = wpool.tile([128, DB, 2 * HD], F32R)
        wo_sb = wpool.tile([128, HPC, D], F32R)

        # weight DMAs, split across two queues so x(tb0) isn't starved:
        # wq alternates sync/scalar; wkv likewise (behind wq); wo on scalar.
        for kb in range(DB):
            eng = nc.sync if kb % 2 == 0 else nc.scalar
            eng.dma_start(out=wq_sb[:, kb, :],
                          in_=wqT[kb * 128:(kb + 1) * 128, :])

        # ---------------- Phase 1: QKV projections + norm/rope -------------
        with tc.tile_pool(name="xp", bufs=3) as xp, \
             tc.tile_pool(name="cs", bufs=2) as cs, \
             tc.tile_pool(name="rp", bufs=2) as rp, \
             tc.tile_pool(name="dg", bufs=2) as dgp, \
             tc.tile_pool(name="qps", bufs=2, space="PSUM") as qps_pool, \
             tc.tile_pool(name="kvps", bufs=2, space="PSUM") as kvps_pool, \
             tc.tile_pool(name="trps", bufs=4, space="PSUM") as trps:

            pend_tr = []          # deferred transpose emissions

            def flush_tr():
                for emit in pend_tr:
                    emit()
                del pend_tr[:]

            for tb in range(SB):
                ts = slice(tb * 128, (tb + 1) * 128)

                x_h = []
                for half in range(2):
                    xh = xp.tile([128, 8, 128], F32R, tag=f"x{half}")
                    nc.sync.dma_start(
                        out=xh, in_=xP[tb, :, half * 8:(half + 1) * 8, :])
                    x_h.append(xh)
                cs_t = cs.tile([128, 4, HD], F32, tag="cs")
                nc.sync.dma_start(out=cs_t, in_=cs4[ts, :, :])
                cq, sq = cs_t[:, 0, :], cs_t[:, 1, :]
                ck, sk = cs_t[:, 2, :], cs_t[:, 3, :]
                if tb == 0:
                    for kb in range(DB):
                        eng = nc.sync if kb % 2 == 0 else nc.scalar
                        eng.dma_start(out=wkv_sb[:, kb, :],
                                      in_=wkvT[kb * 128:(kb + 1) * 128, :])
                if tb == 1:
                    for hb in range(HPC):
                        nc.scalar.dma_start(
                            out=wo_sb[:, hb, :],
                            in_=woT[hb * 128:(hb + 1) * 128, :])

                q_ps = qps_pool.tile([128, HPC * HD], F32, tag="q")
                kv_ps = kvps_pool.tile([128, 2 * HD], F32, tag="kv")
                for kb in range(DB):
                    xb = x_h[kb // 8][:, kb % 8, :]
                    nc.tensor.matmul(q_ps, lhsT=xb, rhs=wq_sb[:, kb, :],
                                     start=(kb == 0), stop=(kb == DB - 1))
                    nc.tensor.matmul(kv_ps, lhsT=xb, rhs=wkv_sb[:, kb, :],
                                     start=(kb == 0), stop=(kb == DB - 1))
                # previous tb's transposes keep the PE busy while this tb's
                # rope/rms runs on DVE/ACT/GPSIMD.
                flush_tr()

                # --- RMS stats on ACT (Square accumulates sum along free) ---
                ms = rp.tile([128, 8], F32, tag="ms")
                scr = rp.tile([128, HD], F32, tag="scr")
                for h in range(HPC):
                    nc.scalar.activation(
                        out=scr, in_=q_ps[:, h * HD:(h + 1) * HD],
                        func=AF.Square, accum_out=ms[:, h:h + 1])
                k_sb = rp.tile([128, HD], F32, tag="k_sb")
                nc.scalar.copy(out=k_sb, in_=kv_ps[:, 0:HD])
                nc.scalar.copy(out=v_all[:, tb, :], in_=kv_ps[:, HD:])
                nc.scalar.activation(out=scr, in_=k_sb, func=AF.Square,
                                     accum_out=ms[:, HPC:HPC + 1])
                srq = rp.tile([128, 8], F32, tag="srq")
                nc.scalar.activation(out=srq[:, 0:HPC + 1],
                                     in_=ms[:, 0:HPC + 1], func=AF.Sqrt,
                                     bias=eps_t, scale=1.0 / HD)
                rq = rp.tile([128, 8], F32, tag="rq")
                nc.vector.reciprocal(out=rq[:, 0:HPC + 1],
                                     in_=srq[:, 0:HPC + 1])

                # --- rope q: wide over all 4 heads, straight from PSUM ---
                # cos/sin broadcast across heads via stride-0 views
                h2 = HD // 2
                cq4 = bass.AP(tensor=cq.tensor, offset=cq.offset,
                              ap=[list(cq.ap[0]), [0, HPC]] + list(cq.ap[1:]))
                sq4 = bass.AP(tensor=sq.tensor, offset=sq.offset,
                              ap=[list(sq.ap[0]), [0, HPC]] + list(sq.ap[1:]))
                q4 = q_ps[:, :].rearrange("pp (h d) -> pp h d", h=HPC)
                t14 = rp.tile([128, HPC, HD], F32, tag="t14")
                nc.vector.tensor_mul(t14, q4, cq4)
                t2p4 = rp.tile([128, HPC, HD], F32, tag="t2p4")
                # low half: q_hi * (-sin_lo)  (sign folded on host)
                nc.vector.tensor_mul(
                    t2p4[:, :, 0:h2], q4[:, :, h2:HD], sq4[:, :, 0:h2])
                nc.vector.tensor_mul(
                    t2p4[:, :, h2:HD], q4[:, :, 0:h2], sq4[:, :, h2:HD])
                qr = rp.tile([128, HPC, HD], F32R, tag="qr")
                nc.vector.tensor_add(qr, t14, t2p4)

                # --- rope k on GPSIMD (SBUF-only engine) ---
                kr = rp.tile([128, HD], F32R, tag="kr")
                t1k = rp.tile([128, HD], F32, tag="t1k")
                nc.gpsimd.tensor_mul(t1k, k_sb, ck)
                t2k = rp.tile([128, HD], F32, tag="t2k")
                nc.gpsimd.tensor_mul(t2k[:, 0:h2], k_sb[:, h2:HD], sk[:, 0:h2])
                nc.gpsimd.tensor_mul(t2k[:, h2:HD], k_sb[:, 0:h2], sk[:, h2:HD])
                nc.gpsimd.tensor_add(kr, t1k, t2k)

                # --- diag(rq) tiles on GPSIMD ---
                dgs = []
                for j in range(HPC + 1):
                    dg = dgp.tile([128, 128], F32R, tag=f"dg{j}")
                    nc.vector.tensor_scalar_mul(dg, ident, rq[:, j:j + 1])
                    dgs.append(dg)

                # --- transposes (deferred into next tb's PE slot):
                #     qT = qr.T @ diag(rq) applies the RMS scale for free ---
                def emit_tr(tb=tb, ts=ts, qr=qr, kr=kr, dgs=dgs):
                    for h in range(HPC):
                        tr_ps = trps.tile([128, 128], F32, tag="tr")
                        nc.tensor.matmul(tr_ps, lhsT=qr[:, h, :],
                                         rhs=dgs[h], start=True, stop=True)
                        if h >= 2:
                            nc.scalar.copy(out=qT_all[:, h, ts], in_=tr_ps)
                        else:
                            nc.vector.tensor_copy(out=qT_all[:, h, ts],
                                                  in_=tr_ps)
                    tr_ps = trps.tile([128, 128], F32, tag="tr")
                    nc.tensor.matmul(tr_ps, lhsT=kr, rhs=dgs[HPC],
                                     start=True, stop=True)
                    nc.vector.tensor_copy(out=kT_all[:, ts], in_=tr_ps)
                pend_tr.append(emit_tr)
            flush_tr()

        # -------- Phase 2+3: attention (group-major) fused with WO ---------
        # Deferred-emission software pipeline, one global pair-iteration
        # counter.  Per pair: scores MMs -> ACT exp -> (post-exp masking on
        # the SBUF exp tile, so no DVE op ever gates the exp) ; sums/AV
        # matmuls drain TWO pair-iterations later so the PE never waits on
        # the ACT exp round-trip.  Softmax normalization runs entirely on
        # ACT + DMA: 1/Z = exp(-ln Z) with a DRAM round-trip broadcast
        # (Ln and Exp share one activation table set).  WO for group g is
        # emitted after head (g+1, 0), giving the last head's
        # normalization a full head of slack; its y tiles go out via DMA
        # straight from PSUM.  PSUM: scores/y 3x2 banks + sums 1 + AV 1.
        n_groups = SB // 4
        with tc.tile_pool(name="exp", bufs=3) as exp_pool, \
             tc.tile_pool(name="attn", bufs=2) as attn_pool, \
             tc.tile_pool(name="avsb", bufs=4) as avsb_pool, \
             tc.tile_pool(name="rcpp", bufs=2) as rcp_pool, \
             tc.tile_pool(name="s_ps", bufs=3, space="PSUM") as s_ps_pool, \
             tc.tile_pool(name="sm_ps", bufs=1, space="PSUM") as sm_ps_pool, \
             tc.tile_pool(name="av_ps", bufs=1, space="PSUM") as av_ps_pool:

            it = [0]              # global pair-iteration counter
            sum_q = []            # (emit_fn,) sums/AV, drained at depth 2
            fin_q = []            # (ready_iter, emit_fn) deferred finishers
            wo_q = []             # deferred WO group emissions

            def drain(keep_sums=2):
                # sums first: a finisher reads av/sm tiles, so the deferred
                # matmuls that write them must be emitted before it
                while len(sum_q) > keep_sums:
                    sum_q.pop(0)()
                while fin_q and fin_q[0][0] <= it[0]:
                    fin_q.pop(0)[1]()

            for g in range(n_groups):
                rbs = list(range(g * 4, g * 4 + 4))
                eblks = [_ext(rb, p) // 128 for rb in rbs]
                gmax = max(eblks)
                qsl = slice(g * 512, (g + 1) * 512)
                attnT = attn_pool.tile([128, HPC, 512], F32R, tag="attnT")

                for h in range(HPC):
                    if h == 1 and wo_q:
                        wo_q.pop(0)()     # WO(g-1): its attnT is long ready
                    sm_ps = sm_ps_pool.tile([1, 512], F32, tag="sm")
                    av_ps = av_ps_pool.tile([128, 512], F32, tag="av")

                    for kbp in range(0, gmax, 2):
                        npair = min(2, gmax - kbp)
                        s_ps = s_ps_pool.tile([128, 1024], F32, tag="s")
                        for j in range(npair):
                            kb = kbp + j
                            nc.tensor.matmul(
                                s_ps[:, j * 512:j * 512 + 512],
                                lhsT=kT_all[:, kb * 128:(kb + 1) * 128],
                                rhs=qT_all[:, h, qsl],
                                start=True, stop=True)
                        ex = exp_pool.tile([128, 2, 512], F32R, tag="ex")
                        nc.scalar.activation(
                            out=ex[:, 0:npair, :],
                            in_=s_ps[:, 0:npair * 512],
                            func=AF.Exp, scale=SOFT_SCALE)
                        # post-exp masking on SBUF (never gates the exp):
                        # zero q-columns whose extent <= kb (a prefix of the
                        # group) and multiply the causal diagonal block by
                        # the 0/1 lower-triangle mask.
                        for j in range(npair):
                            kb = kbp + j
                            jm = sum(1 for e in eblks if e <= kb)
                            if jm > 0:
                                # memset can't write f32r; scale by 0 instead
                                nc.vector.tensor_scalar_mul(
                                    ex[:, j, 0:jm * 128],
                                    ex[:, j, 0:jm * 128], 0.0)
                            ri_d = kb - g * 4
                            if 0 <= ri_d < 4 and kb * 128 >= p \
                                    and eblks[ri_d] == kb + 1:
                                od = ri_d * 128
                                nc.vector.tensor_mul(
                                    ex[:, j, od:od + 128],
                                    ex[:, j, od:od + 128], dmask_sb)

                        def emit_sums(ex=ex, npair=npair, kbp=kbp,
                                      gmax=gmax, sm_ps=sm_ps, av_ps=av_ps):
                            for j in range(npair):
                                kb = kbp + j
                                exj = ex[:, j, :]
                                nc.tensor.matmul(sm_ps, lhsT=ones_col,
                                                 rhs=exj, start=(kb == 0),
                                                 stop=(kb == gmax - 1))
                                nc.tensor.matmul(av_ps,
                                                 lhsT=v_all[:, kb, :],
                                                 rhs=exj, start=(kb == 0),
                                                 stop=(kb == gmax - 1))
                        sum_q.append(emit_sums)
                        it[0] += 1
                        drain(keep_sums=2)

                    # head finishers, deferred 2 pair-iterations:
                    #   stage A: av -> SBUF (ACT), lnZ (ACT), lnZ -> DRAM
                    #   stage B (2 more iters): bcast read, 1/Z = exp(-lnZ)
                    #            (ACT), attnT = av * 1/Z (DVE)
                    slot = g * HPC + h
                    def fin_a(slot=slot, sm_ps=sm_ps, av_ps=av_ps,
                              h=h, attnT=attnT):
                        lnz = rcp_pool.tile([1, 512], F32, tag="lnz")
                        nc.scalar.activation(out=lnz, in_=sm_ps, func=AF.Ln)
                        nc.sync.dma_start(out=rcp_scr[slot:slot + 1, :],
                                          in_=lnz)
                        av_sb = avsb_pool.tile([128, 512], F32, tag="av_sb")
                        nc.vector.tensor_copy(out=av_sb, in_=av_ps)
                        def fin_b(slot=slot, av_sb=av_sb, h=h, attnT=attnT):
                            lbc = rcp_pool.tile([128, 512], F32, tag="lbc")
                            drap = rcp_scr[slot:slot + 1, :]
                            bcast = bass.AP(tensor=drap.tensor,
                                            offset=drap.offset,
                                            ap=[[0, 128]] + list(drap.ap[1:]))
                            nc.sync.dma_start(out=lbc, in_=bcast)
                            rbc = rcp_pool.tile([128, 512], F32, tag="rbc")
                            nc.scalar.activation(out=rbc, in_=lbc,
                                                 func=AF.Exp, scale=-1.0)
                            nc.vector.tensor_mul(attnT[:, h, :], av_sb, rbc)
                        fin_q.append((it[0] + 2, fin_b))
                    fin_q.append((it[0] + 2, fin_a))

                # ---- WO for this token chunk, deferred one head ----
                def emit_wo(g=g, attnT=attnT):
                    for db in range(DB):
                        y_ps = s_ps_pool.tile([128, 1024], F32, tag="s")
                        for hb in range(HPC):
                            nc.tensor.matmul(
                                y_ps[:, 0:512],
                                lhsT=wo_sb[:, hb, db * 128:(db + 1) * 128],
                                rhs=attnT[:, hb, :],
                                start=(hb == 0), stop=(hb == HPC - 1))
                        y_sb = avsb_pool.tile([128, 512], F32, tag="y_sb")
                        nc.vector.tensor_copy(out=y_sb, in_=y_ps[:, 0:512])
                        nc.sync.dma_start(
                            out=yT[db * 128:(db + 1) * 128,
                                   g * 512:(g + 1) * 512],
                            in_=y_sb)
                wo_q.append(emit_wo)

            while sum_q or fin_q:
                it[0] += 1
                drain(keep_sums=0)
            while wo_q:
                wo_q.pop(0)()

    if legalize:
        _legalize_waits(nc)
    return nc


def _prep_inputs(x, cos, sin, wq, wk, wv, wo, q_gamma, k_gamma, p):
    """Build the 8 per-core input maps."""
    cos2 = np.asarray(cos, np.float32).reshape(S, HD)
    sin2 = np.asarray(sin, np.float32).reshape(S, HD)
    qg = np.asarray(q_gamma, np.float32)
    kg = np.asarray(k_gamma, np.float32)
    h = HD // 2
    qg_rot = np.concatenate([qg[h:], qg[:h]])
    kg_rot = np.concatenate([kg[h:], kg[:h]])
    cos_q = cos2 * qg
    sin_q = sin2 * qg_rot
    cos_k = cos2 * kg
    sin_k = sin2 * kg_rot
    # fold the rotate-half sign into the low halves of sin
    sin_q[:, :h] *= -1.0
    sin_k[:, :h] *= -1.0
    # pack [cos_q | sin_q | cos_k | sin_k] so each tb is one 2KB-row DMA
    cs4 = np.ascontiguousarray(
        np.stack([cos_q, sin_q, cos_k, sin_k], axis=1))

    ii = np.arange(128)
    dmask = (ii[:, None] <= ii[None, :]).astype(np.float32)

    x = np.asarray(x, np.float32)
    wq = np.asarray(wq, np.float32)
    wk = np.asarray(wk, np.float32)
    wv = np.asarray(wv, np.float32)
    wo = np.asarray(wo, np.float32)

    # xP[tb, pp, kb, ti] = x[b, tb*128+ti, kb*128+pp]: the per-tb SBUF
    # x tile loads become 128 contiguous 4KB descriptors instead of 1024
    # 512B ones.
    xP = [np.ascontiguousarray(
        x[b].reshape(SB, 128, DB, 128).transpose(0, 3, 2, 1))
        for b in range(B)]
    in_maps = []
    for c in range(N_CORES):
        b, g = divmod(c, N_CORES // B)
        h0 = g * HPC
        kv = h0 // (NH // KVH)
        wqTc = np.ascontiguousarray(wq[h0 * HD:(h0 + HPC) * HD, :].T)
        wkvTc = np.ascontiguousarray(
            np.concatenate([wk[kv * HD:(kv + 1) * HD, :],
                            wv[kv * HD:(kv + 1) * HD, :]], axis=0).T)
        woTc = np.ascontiguousarray(wo[:, h0 * HD:(h0 + HPC) * HD].T)
        in_maps.append({
            "xP": xP[b], "wqT": wqTc, "wkvT": wkvTc, "woT": woTc,
            "cs4": cs4, "dmask": dmask,
        })
    return in_maps


def _gather(results):
    y = np.zeros((B, S, D), dtype=np.float32)
    for c in range(N_CORES):
        b = c // (N_CORES // B)
        y[b] += results[c]["yT"].T
    return y


def kernel(x, cos, sin, wq, wk, wv, wo, q_gamma, k_gamma, signal_token_num):
    p = int(signal_token_num)
    assert p % 128 == 0 and 0 <= p <= S, f"unsupported signal_token_num {p}"

    nc = build_core_kernel(p)
    in_maps = _prep_inputs(x, cos, sin, wq, wk, wv, wo, q_gamma, k_gamma, p)
    res = run_bass_kernel_spmd(nc, in_maps, list(range(N_CORES)))
    return _gather(res.results)


def _install_ntff_hook():
    """The container's antenv lacks axon_hooks; replicate the boot-time NTFF
    profile hook (ctypes into libaxon_pjrt.so) and register the module."""
    import sys
    import types
    import ctypes
    import contextlib

    if "antenv.axon_hooks" in sys.modules:
        return
    so_path = "/opt/axon/libaxon_pjrt.so"
    lib = ctypes.CDLL(so_path)
    if not hasattr(lib, "axon_start_nrt_profile"):
        return
    lib.axon_start_nrt_profile.argtypes = [
        ctypes.POINTER(ctypes.c_int64), ctypes.c_size_t]
    lib.axon_start_nrt_profile.restype = ctypes.c_int64
    lib.axon_stop_nrt_profile.argtypes = [ctypes.c_char_p]
    lib.axon_stop_nrt_profile.restype = ctypes.c_int64

    @contextlib.contextmanager
    def _hook(output_dir, device_ids):
        import jax
        jax.devices()
        if device_ids:
            ids = (ctypes.c_int64 * len(device_ids))(*device_ids)
            rc = lib.axon_start_nrt_profile(ids, len(device_ids))
        else:
            rc = lib.axon_start_nrt_profile(None, 0)
        if rc != 0:
            raise RuntimeError(f"axon_start_nrt_profile rc={rc}")
        try:
            yield
        finally:
            n = lib.axon_stop_nrt_profile(str(output_dir).encode())
            print(f"profile: {n} file(s) written to {output_dir}")

    import antenv
    mod = types.ModuleType("antenv.axon_hooks")
    mod.get_axon_ntff_profile_hook = lambda: _hook
    mod.set_axon_ntff_profile_hook = lambda h: None
    sys.modules["antenv.axon_hooks"] = mod
    antenv.axon_hooks = mod


def profile_once(inputs):
    """Run once with NTFF tracing; return max per-core exec time in ns."""
    import concourse.bass_utils as bu
    bu.upload_artifacts = lambda tmpdir: ""   # no bucket access here
    _install_ntff_hook()
    p = int(inputs["signal_token_num"])
    nc = build_core_kernel(p)
    in_maps = _prep_inputs(
        inputs["x"], inputs["cos"], inputs["sin"], inputs["wq"], inputs["wk"],
        inputs["wv"], inputs["wo"], inputs["q_gamma"], inputs["k_gamma"], p)
    try:
        res = bu.run_bass_kernel_spmd(nc, in_maps, list(range(N_CORES)),
                                      trace=True,
                                      trace_cores=list(range(N_CORES)))
        return res.exec_time_ns
    except Exception as e:
        print(f"profile failed: {type(e).__name__}: {e}")
        return None


# revision 16
# speedup vs baseline: 1.6108x; 1.0286x over previous
"""Trainium2 Bass kernel for GQA attention with QK-RMSNorm, RoPE and a
bidirectional-prefix + causal mask (sparse_attention problem).

Reference computation (fp32):
  xq = x @ wq.T; xk = x @ wk.T; xv = x @ wv.T   (per-head RMSNorm on q,k)
  rope(q), rope(k); repeat kv heads 8x
  scores = q k^T / sqrt(128); mask = causal OR (i<p & j<p)
  out = softmax(scores) @ v;  y = out @ wo.T

Sharding: 8 cores = 2 batches x 4 head-groups (4 query heads each, sharing
one KV head).  Each core computes a partial y^T (its 4 heads' contribution);
the host sums the 4 partials per batch and transposes back.

v2 design notes (vs the first working version):
  * No fp32->fp32r staging copies: matmul-consumed DRAM tensors and SBUF
    tiles are declared float32r (same 32-bit layout host-side) so DMA
    lands them directly; engine-written operands write f32r natively.
  * RMSNorm sum-of-squares runs on the Scalar engine (Square + accum_out),
    the per-token 1/sqrt scale is applied FOR FREE by the PE transpose:
    instead of transposing with the identity, we transpose with
    diag(rq) so qT = q.T @ diag(rq) lands pre-scaled.
  * RoPE sign is folded into sin (host negates the low half), so rope is
    3 wide DVE multiplies + 1 add across all 4 heads at once, reading the
    projection results directly from PSUM.
  * K-path rope + the diag(rq) builds run on the otherwise-idle GPSIMD.
  * Softmax: exp on ACT (per 2-block pair), row sums via a ones-matmul,
    reciprocal on DVE, and the [1,512] -> [128,512] broadcast is a rank-1
    PE matmul (ones outer product) instead of a DRAM round-trip.
  * Single PE instruction stream ordered so the PE never has a long gap
    (projection MMs -> previous tb's transposes -> ... -> attention),
    keeping the HAM clock-gate at 8/8.
  * ACT program order is strictly {Square,Copy,Sqrt} then {Exp,Copy}, so
    exactly two activation-table loads happen.

TRN2 ISA allows ONE sync-wait per instruction and walrus does not split
multi-wait instructions, so `_legalize_waits` rewrites the emitted BIR,
moving excess waits onto preceding same-engine NoOps.
"""
import math
import numpy as np
from contextlib import ExitStack

import bass_rust
import concourse.bass as bass
import concourse.mybir as mybir
import concourse.tile as tile
from concourse.bass_utils import run_bass_kernel_spmd
from concourse.masks import make_identity

F32 = mybir.dt.float32
F32R = mybir.dt.float32r
BF16 = mybir.dt.bfloat16
AF = mybir.ActivationFunctionType

B, S, D = 2, 2048, 2048
NH, KVH, HD = 16, 2, 128
HPC = 4                      # query heads per core
N_CORES = 8
EPS = 1e-6
SOFT_SCALE = 1.0 / math.sqrt(HD)
NEG = -1.0e30

SB = S // 128                # 16 token blocks
DB = D // 128                # 16 contraction blocks

_lgw_counter = [0]


def _legalize_waits(nc, cap=1):
    """Move all-but-`cap` sync waits of every instruction onto preceding
    same-engine NoOps (TRN2 EVENTS block has a single wait slot)."""
    for fn in nc.m.functions:
        for blk in fn.blocks:
            out = []
            changed = False
            for inst in blk.instructions:
                si = inst.sync_info
                waits = list(si.on_wait) if si is not None and si.on_wait else []
                if len(waits) > cap:
                    changed = True
                    move, keep = waits[:-cap], waits[-cap:]
                    for w in move:
                        n = bass_rust.InstNoOp(name=f"LGW-{_lgw_counter[0]}")
                        _lgw_counter[0] += 1
                        n.engine = inst.engine
                        n.sync_info = mybir.SyncInfo(on_wait=[w], on_update=[])
                        out.append(n)
                    inst.sync_info = mybir.SyncInfo(
                        on_wait=keep, on_update=list(si.on_update or []))
                out.append(inst)
            if changed:
                blk.instructions = out
    return nc


def _ext(rb, p):
    """Key extent attended by query row-block rb (rows rb*128 .. rb*128+127)."""
    lo, hi = rb * 128, (rb + 1) * 128
    if hi <= p:
        return p              # prefix rows attend the full prefix [0, p)
    return hi                 # causal rows attend [0, hi), diag-masked


def build_core_kernel(p, legalize=True):
    """One SPMD program; per-core behavior differs only via input data."""
    nc = bass.Bass()

    xP = nc.dram_tensor("xP", [SB, 128, DB, 128], F32R, kind="ExternalInput")
    xB = nc.dram_tensor("xB", [SB, 128, DB, 128], BF16, kind="ExternalInput")
    wqT = nc.dram_tensor("wqT", [D, HPC * HD], F32R, kind="ExternalInput")
    wkvT = nc.dram_tensor("wkvT", [D, 2 * HD], BF16, kind="ExternalInput")
    woT = nc.dram_tensor("woT", [HPC * HD, D], F32R, kind="ExternalInput")
    cs4 = nc.dram_tensor("cs4", [S, 4, HD], F32, kind="ExternalInput")
    dmask = nc.dram_tensor("dmask", [128, 128], F32, kind="ExternalInput")
    rcp_scr = nc.dram_tensor("rcp_scr", [SB * HPC, 512], F32)
    yT = nc.dram_tensor("yT", [D, S], F32, kind="ExternalOutput")

    with tile.TileContext(nc) as tc, ExitStack() as octx:
        const = octx.enter_context(tc.tile_pool(name="const", bufs=1))
        ident = const.tile([128, 128], F32)
        make_identity(nc, ident)
        dmask_sb = const.tile([128, 128], F32)
        nc.sync.dma_start(out=dmask_sb, in_=dmask[:, :])
        eps_t = const.tile([128, 1], F32)
        nc.vector.memset(eps_t, EPS)
        ones_f = const.tile([128, 1], F32)
        nc.vector.memset(ones_f, 1.0)
        ones_col = const.tile([128, 1], F32R)
        nc.vector.tensor_copy(out=ones_col, in_=ones_f)

        qkv = octx.enter_context(tc.tile_pool(name="qkv", bufs=1))
        qT_all = qkv.tile([128, HPC, S], F32R)        # [hd, h, tok]
        kT_all = qkv.tile([128, S], F32R)             # [hd, tok]
        v_all = qkv.tile([128, SB, HD], F32R)         # [tok(P), tb, hd]

        wpool = octx.enter_context(tc.tile_pool(name="w", bufs=1))
        wq_sb = wpool.tile([128, DB, HPC * HD], F32R)
        wkv_sb = wpool.tile([128, DB, 2 * HD], BF16)
        wo_sb = wpool.tile([128, HPC, D], F32R)

        # weight DMAs, split across two queues so x(tb0) isn't starved:
        # wq alternates sync/scalar; wkv likewise (behind wq); wo on scalar.
        for kb in range(DB):
            eng = nc.sync if kb % 2 == 0 else nc.scalar
            eng.dma_start(out=wq_sb[:, kb, :],
                          in_=wqT[kb * 128:(kb + 1) * 128, :])

        # ---------------- Phase 1: QKV projections + norm/rope -------------
        with tc.tile_pool(name="xp", bufs=3) as xp, \
             tc.tile_pool(name="cs", bufs=2) as cs, \
             tc.tile_pool(name="rp", bufs=2) as rp, \
             tc.tile_pool(name="dg", bufs=2) as dgp, \
             tc.tile_pool(name="qps", bufs=2, space="PSUM") as qps_pool, \
             tc.tile_pool(name="kvps", bufs=2, space="PSUM") as kvps_pool, \
             tc.tile_pool(name="trps", bufs=4, space="PSUM") as trps:

            pend_tr = []          # deferred transpose emissions

            def flush_tr():
                for emit in pend_tr:
                    emit()
                del pend_tr[:]

            for tb in range(SB):
                ts = slice(tb * 128, (tb + 1) * 128)

                x_h, xb_h = [], []
                for half in range(2):
                    xh = xp.tile([128, 8, 128], F32R, tag=f"x{half}")
                    nc.sync.dma_start(
                        out=xh, in_=xP[tb, :, half * 8:(half + 1) * 8, :])
                    x_h.append(xh)
                    xbh = xp.tile([128, 8, 128], BF16, tag=f"xb{half}")
                    nc.sync.dma_start(
                        out=xbh, in_=xB[tb, :, half * 8:(half + 1) * 8, :])
                    xb_h.append(xbh)
                cs_t = cs.tile([128, 4, HD], F32, tag="cs")
                nc.sync.dma_start(out=cs_t, in_=cs4[ts, :, :])
                cq, sq = cs_t[:, 0, :], cs_t[:, 1, :]
                ck, sk = cs_t[:, 2, :], cs_t[:, 3, :]
                if tb == 0:
                    for kb in range(DB):
                        eng = nc.sync if kb % 2 == 0 else nc.scalar
                        eng.dma_start(out=wkv_sb[:, kb, :],
                                      in_=wkvT[kb * 128:(kb + 1) * 128, :])
                if tb == 1:
                    for hb in range(HPC):
                        nc.scalar.dma_start(
                            out=wo_sb[:, hb, :],
                            in_=woT[hb * 128:(hb + 1) * 128, :])

                q_ps = qps_pool.tile([128, HPC * HD], F32, tag="q")
                kv_ps = kvps_pool.tile([128, 2 * HD], F32, tag="kv")
                for kb in range(DB):
                    nc.tensor.matmul(q_ps, lhsT=x_h[kb // 8][:, kb % 8, :],
                                     rhs=wq_sb[:, kb, :],
                                     start=(kb == 0), stop=(kb == DB - 1))
                    nc.tensor.matmul(kv_ps, lhsT=xb_h[kb // 8][:, kb % 8, :],
                                     rhs=wkv_sb[:, kb, :],
                                     start=(kb == 0), stop=(kb == DB - 1))
                # previous tb's transposes keep the PE busy while this tb's
                # rope/rms runs on DVE/ACT/GPSIMD.
                flush_tr()

                # --- RMS stats on ACT (Square accumulates sum along free) ---
                ms = rp.tile([128, 8], F32, tag="ms")
                scr = rp.tile([128, HD], F32, tag="scr")
                for h in range(HPC):
                    nc.scalar.activation(
                        out=scr, in_=q_ps[:, h * HD:(h + 1) * HD],
                        func=AF.Square, accum_out=ms[:, h:h + 1])
                k_sb = rp.tile([128, HD], F32, tag="k_sb")
                nc.scalar.copy(out=k_sb, in_=kv_ps[:, 0:HD])
                nc.scalar.copy(out=v_all[:, tb, :], in_=kv_ps[:, HD:])
                nc.scalar.activation(out=scr, in_=k_sb, func=AF.Square,
                                     accum_out=ms[:, HPC:HPC + 1])
                srq = rp.tile([128, 8], F32, tag="srq")
                nc.scalar.activation(out=srq[:, 0:HPC + 1],
                                     in_=ms[:, 0:HPC + 1], func=AF.Sqrt,
                                     bias=eps_t, scale=1.0 / HD)
                rq = rp.tile([128, 8], F32, tag="rq")
                nc.vector.reciprocal(out=rq[:, 0:HPC + 1],
                                     in_=srq[:, 0:HPC + 1])

                # --- rope q: wide over all 4 heads, straight from PSUM ---
                # cos/sin broadcast across heads via stride-0 views
                h2 = HD // 2
                cq4 = bass.AP(tensor=cq.tensor, offset=cq.offset,
                              ap=[list(cq.ap[0]), [0, HPC]] + list(cq.ap[1:]))
                sq4 = bass.AP(tensor=sq.tensor, offset=sq.offset,
                              ap=[list(sq.ap[0]), [0, HPC]] + list(sq.ap[1:]))
                q4 = q_ps[:, :].rearrange("pp (h d) -> pp h d", h=HPC)
                t14 = rp.tile([128, HPC, HD], F32, tag="t14")
                nc.vector.tensor_mul(t14, q4, cq4)
                t2p4 = rp.tile([128, HPC, HD], F32, tag="t2p4")
                # low half: q_hi * (-sin_lo)  (sign folded on host)
                nc.vector.tensor_mul(
                    t2p4[:, :, 0:h2], q4[:, :, h2:HD], sq4[:, :, 0:h2])
                nc.vector.tensor_mul(
                    t2p4[:, :, h2:HD], q4[:, :, 0:h2], sq4[:, :, h2:HD])
                qr = rp.tile([128, HPC, HD], F32R, tag="qr")
                nc.vector.tensor_add(qr, t14, t2p4)

                # --- rope k on GPSIMD (SBUF-only engine) ---
                kr = rp.tile([128, HD], F32R, tag="kr")
                t1k = rp.tile([128, HD], F32, tag="t1k")
                nc.gpsimd.tensor_mul(t1k, k_sb, ck)
                t2k = rp.tile([128, HD], F32, tag="t2k")
                nc.gpsimd.tensor_mul(t2k[:, 0:h2], k_sb[:, h2:HD], sk[:, 0:h2])
                nc.gpsimd.tensor_mul(t2k[:, h2:HD], k_sb[:, 0:h2], sk[:, h2:HD])
                nc.gpsimd.tensor_add(kr, t1k, t2k)

                # --- diag(rq) tiles on GPSIMD ---
                dgs = []
                for j in range(HPC + 1):
                    dg = dgp.tile([128, 128], F32R, tag=f"dg{j}")
                    nc.vector.tensor_scalar_mul(dg, ident, rq[:, j:j + 1])
                    dgs.append(dg)

                # --- transposes (deferred into next tb's PE slot):
                #     qT = qr.T @ diag(rq) applies the RMS scale for free ---
                def emit_tr(tb=tb, ts=ts, qr=qr, kr=kr, dgs=dgs):
                    for h in range(HPC):
                        tr_ps = trps.tile([128, 128], F32, tag="tr")
                        nc.tensor.matmul(tr_ps, lhsT=qr[:, h, :],
                                         rhs=dgs[h], start=True, stop=True)
                        if h >= 2:
                            nc.scalar.copy(out=qT_all[:, h, ts], in_=tr_ps)
                        else:
                            nc.vector.tensor_copy(out=qT_all[:, h, ts],
                                                  in_=tr_ps)
                    tr_ps = trps.tile([128, 128], F32, tag="tr")
                    nc.tensor.matmul(tr_ps, lhsT=kr, rhs=dgs[HPC],
                                     start=True, stop=True)
                    nc.vector.tensor_copy(out=kT_all[:, ts], in_=tr_ps)
                pend_tr.append(emit_tr)
            flush_tr()

        # -------- Phase 2+3: attention (group-major) fused with WO ---------
        # Deferred-emission software pipeline, one global pair-iteration
        # counter.  Per pair: scores MMs -> ACT exp -> (post-exp masking on
        # the SBUF exp tile, so no DVE op ever gates the exp) ; sums/AV
        # matmuls drain TWO pair-iterations later so the PE never waits on
        # the ACT exp round-trip.  Softmax normalization runs entirely on
        # ACT + DMA: 1/Z = exp(-ln Z) with a DRAM round-trip broadcast
        # (Ln and Exp share one activation table set).  WO for group g is
        # emitted after head (g+1, 0), giving the last head's
        # normalization a full head of slack; its y tiles go out via DMA
        # straight from PSUM.  PSUM: scores/y 3x2 banks + sums 1 + AV 1.
        n_groups = SB // 4
        with tc.tile_pool(name="exp", bufs=3) as exp_pool, \
             tc.tile_pool(name="attn", bufs=2) as attn_pool, \
             tc.tile_pool(name="avsb", bufs=4) as avsb_pool, \
             tc.tile_pool(name="rcpp", bufs=2) as rcp_pool, \
             tc.tile_pool(name="s_ps", bufs=3, space="PSUM") as s_ps_pool, \
             tc.tile_pool(name="sm_ps", bufs=1, space="PSUM") as sm_ps_pool, \
             tc.tile_pool(name="av_ps", bufs=1, space="PSUM") as av_ps_pool:

            it = [0]              # global pair-iteration counter
            sum_q = []            # (emit_fn,) sums/AV, drained at depth 2
            fin_q = []            # (ready_iter, emit_fn) deferred finishers
            wo_q = []             # deferred WO group emissions

            def drain(keep_sums=2):
                # sums first: a finisher reads av/sm tiles, so the deferred
                # matmuls that write them must be emitted before it
                while len(sum_q) > keep_sums:
                    sum_q.pop(0)()
                while fin_q and fin_q[0][0] <= it[0]:
                    fin_q.pop(0)[1]()

            for g in range(n_groups):
                rbs = list(range(g * 4, g * 4 + 4))
                eblks = [_ext(rb, p) // 128 for rb in rbs]
                gmax = max(eblks)
                qsl = slice(g * 512, (g + 1) * 512)
                attnT = attn_pool.tile([128, HPC, 512], F32R, tag="attnT")

                for h in range(HPC):
                    if h >= 1:
                        for _ in range(min(6, len(wo_q))):
                            wo_q.pop(0)()   # WO(g-1) dbs, spread over heads
                    sm_ps = sm_ps_pool.tile([1, 512], F32, tag="sm")
                    av_ps = av_ps_pool.tile([128, 512], F32, tag="av")

                    for kbp in range(0, gmax, 2):
                        npair = min(2, gmax - kbp)
                        s_ps = s_ps_pool.tile([128, 1024], F32, tag="s")
                        for j in range(npair):
                            kb = kbp + j
                            nc.tensor.matmul(
                                s_ps[:, j * 512:j * 512 + 512],
                                lhsT=kT_all[:, kb * 128:(kb + 1) * 128],
                                rhs=qT_all[:, h, qsl],
                                start=True, stop=True)
                        ex = exp_pool.tile([128, 2, 512], F32R, tag="ex")
                        nc.scalar.activation(
                            out=ex[:, 0:npair, :],
                            in_=s_ps[:, 0:npair * 512],
                            func=AF.Exp, scale=SOFT_SCALE)
                        # post-exp masking on SBUF (never gates the exp):
                        # zero q-columns whose extent <= kb (a prefix of the
                        # group) and multiply the causal diagonal block by
                        # the 0/1 lower-triangle mask.
                        for j in range(npair):
                            kb = kbp + j
                            jm = sum(1 for e in eblks if e <= kb)
                            if jm > 0:
                                # memset can't write f32r; scale by 0 instead
                                nc.vector.tensor_scalar_mul(
                                    ex[:, j, 0:jm * 128],
                                    ex[:, j, 0:jm * 128], 0.0)
                            ri_d = kb - g * 4
                            if 0 <= ri_d < 4 and kb * 128 >= p \
                                    and eblks[ri_d] == kb + 1:
                                od = ri_d * 128
                                nc.vector.tensor_mul(
                                    ex[:, j, od:od + 128],
                                    ex[:, j, od:od + 128], dmask_sb)

                        def emit_sums(ex=ex, npair=npair, kbp=kbp,
                                      gmax=gmax, sm_ps=sm_ps, av_ps=av_ps):
                            for j in range(npair):
                                kb = kbp + j
                                exj = ex[:, j, :]
                                nc.tensor.matmul(sm_ps, lhsT=ones_col,
                                                 rhs=exj, start=(kb == 0),
                                                 stop=(kb == gmax - 1))
                                nc.tensor.matmul(av_ps,
                                                 lhsT=v_all[:, kb, :],
                                                 rhs=exj, start=(kb == 0),
                                                 stop=(kb == gmax - 1))
                        sum_q.append(emit_sums)
                        it[0] += 1
                        drain(keep_sums=2)

                    # head finishers, deferred 2 pair-iterations:
                    #   stage A: av -> SBUF (ACT), lnZ (ACT), lnZ -> DRAM
                    #   stage B (2 more iters): bcast read, 1/Z = exp(-lnZ)
                    #            (ACT), attnT = av * 1/Z (DVE)
                    slot = g * HPC + h
                    def fin_a(slot=slot, sm_ps=sm_ps, av_ps=av_ps,
                              h=h, attnT=attnT):
                        lnz = rcp_pool.tile([1, 512], F32, tag="lnz")
                        nc.scalar.activation(out=lnz, in_=sm_ps, func=AF.Ln)
                        nc.sync.dma_start(out=rcp_scr[slot:slot + 1, :],
                                          in_=lnz)
                        av_sb = avsb_pool.tile([128, 512], F32, tag="av_sb")
                        nc.vector.tensor_copy(out=av_sb, in_=av_ps)
                        def fin_b(slot=slot, av_sb=av_sb, h=h, attnT=attnT):
                            lbc = rcp_pool.tile([128, 512], F32, tag="lbc")
                            drap = rcp_scr[slot:slot + 1, :]
                            bcast = bass.AP(tensor=drap.tensor,
                                            offset=drap.offset,
                                            ap=[[0, 128]] + list(drap.ap[1:]))
                            nc.sync.dma_start(out=lbc, in_=bcast)
                            rbc = rcp_pool.tile([128, 512], F32, tag="rbc")
                            nc.scalar.activation(out=rbc, in_=lbc,
                                                 func=AF.Exp, scale=-1.0)
                            nc.vector.tensor_mul(attnT[:, h, :], av_sb, rbc)
                        fin_q.append((it[0] + 2, fin_b))
                    fin_q.append((it[0] + 2, fin_a))

                # ---- WO for this token chunk, deferred one head and
                #      spread across the next group's heads ----
                def emit_wo_db(g=g, attnT=attnT, db=0):
                    y_ps = s_ps_pool.tile([128, 1024], F32, tag="s")
                    for hb in range(HPC):
                        nc.tensor.matmul(
                            y_ps[:, 0:512],
                            lhsT=wo_sb[:, hb, db * 128:(db + 1) * 128],
                            rhs=attnT[:, hb, :],
                            start=(hb == 0), stop=(hb == HPC - 1))
                    y_sb = avsb_pool.tile([128, 512], F32, tag="y_sb")
                    nc.vector.tensor_copy(out=y_sb, in_=y_ps[:, 0:512])
                    nc.sync.dma_start(
                        out=yT[db * 128:(db + 1) * 128,
                               g * 512:(g + 1) * 512],
                        in_=y_sb)
                for db in range(DB):
                    wo_q.append(lambda g=g, attnT=attnT, db=db:
                                emit_wo_db(g, attnT, db))

            while sum_q or fin_q:
                it[0] += 1
                drain(keep_sums=0)
            while wo_q:
                wo_q.pop(0)()

    if legalize:
        _legalize_waits(nc)
    return nc


def _prep_inputs(x, cos, sin, wq, wk, wv, wo, q_gamma, k_gamma, p):
    """Build the 8 per-core input maps."""
    cos2 = np.asarray(cos, np.float32).reshape(S, HD)
    sin2 = np.asarray(sin, np.float32).reshape(S, HD)
    qg = np.asarray(q_gamma, np.float32)
    kg = np.asarray(k_gamma, np.float32)
    h = HD // 2
    qg_rot = np.concatenate([qg[h:], qg[:h]])
    kg_rot = np.concatenate([kg[h:], kg[:h]])
    cos_q = cos2 * qg
    sin_q = sin2 * qg_rot
    cos_k = cos2 * kg
    sin_k = sin2 * kg_rot
    # fold the rotate-half sign into the low halves of sin
    sin_q[:, :h] *= -1.0
    sin_k[:, :h] *= -1.0
    # pack [cos_q | sin_q | cos_k | sin_k] so each tb is one 2KB-row DMA
    cs4 = np.ascontiguousarray(
        np.stack([cos_q, sin_q, cos_k, sin_k], axis=1))

    ii = np.arange(128)
    dmask = (ii[:, None] <= ii[None, :]).astype(np.float32)

    x = np.asarray(x, np.float32)
    wq = np.asarray(wq, np.float32)
    wk = np.asarray(wk, np.float32)
    wv = np.asarray(wv, np.float32)
    wo = np.asarray(wo, np.float32)

    # xP[tb, pp, kb, ti] = x[b, tb*128+ti, kb*128+pp]: the per-tb SBUF
    # x tile loads become 128 contiguous 4KB descriptors instead of 1024
    # 512B ones.
    xP = [np.ascontiguousarray(
        x[b].reshape(SB, 128, DB, 128).transpose(0, 3, 2, 1))
        for b in range(B)]
    import ml_dtypes
    xB = [xp.astype(ml_dtypes.bfloat16) for xp in xP]
    in_maps = []
    for c in range(N_CORES):
        b, g = divmod(c, N_CORES // B)
        h0 = g * HPC
        kv = h0 // (NH // KVH)
        wqTc = np.ascontiguousarray(wq[h0 * HD:(h0 + HPC) * HD, :].T)
        import ml_dtypes
        wkvTc = np.ascontiguousarray(
            np.concatenate([wk[kv * HD:(kv + 1) * HD, :],
                            wv[kv * HD:(kv + 1) * HD, :]],
                           axis=0).T).astype(ml_dtypes.bfloat16)
        woTc = np.ascontiguousarray(wo[:, h0 * HD:(h0 + HPC) * HD].T)
        in_maps.append({
            "xP": xP[b], "xB": xB[b], "wqT": wqTc, "wkvT": wkvTc,
            "woT": woTc, "cs4": cs4, "dmask": dmask,
        })
    return in_maps


def _gather(results):
    y = np.zeros((B, S, D), dtype=np.float32)
    for c in range(N_CORES):
        b = c // (N_CORES // B)
        y[b] += results[c]["yT"].T
    return y


def kernel(x, cos, sin, wq, wk, wv, wo, q_gamma, k_gamma, signal_token_num):
    p = int(signal_token_num)
    assert p % 128 == 0 and 0 <= p <= S, f"unsupported signal_token_num {p}"

    nc = build_core_kernel(p)
    in_maps = _prep_inputs(x, cos, sin, wq, wk, wv, wo, q_gamma, k_gamma, p)
    res = run_bass_kernel_spmd(nc, in_maps, list(range(N_CORES)))
    return _gather(res.results)


def _install_ntff_hook():
    """The container's antenv lacks axon_hooks; replicate the boot-time NTFF
    profile hook (ctypes into libaxon_pjrt.so) and register the module."""
    import sys
    import types
    import ctypes
    import contextlib

    if "antenv.axon_hooks" in sys.modules:
        return
    so_path = "/opt/axon/libaxon_pjrt.so"
    lib = ctypes.CDLL(so_path)
    if not hasattr(lib, "axon_start_nrt_profile"):
        return
    lib.axon_start_nrt_profile.argtypes = [
        ctypes.POINTER(ctypes.c_int64), ctypes.c_size_t]
    lib.axon_start_nrt_profile.restype = ctypes.c_int64
    lib.axon_stop_nrt_profile.argtypes = [ctypes.c_char_p]
    lib.axon_stop_nrt_profile.restype = ctypes.c_int64

    @contextlib.contextmanager
    def _hook(output_dir, device_ids):
        import jax
        jax.devices()
        if device_ids:
            ids = (ctypes.c_int64 * len(device_ids))(*device_ids)
            rc = lib.axon_start_nrt_profile(ids, len(device_ids))
        else:
            rc = lib.axon_start_nrt_profile(None, 0)
        if rc != 0:
            raise RuntimeError(f"axon_start_nrt_profile rc={rc}")
        try:
            yield
        finally:
            n = lib.axon_stop_nrt_profile(str(output_dir).encode())
            print(f"profile: {n} file(s) written to {output_dir}")

    import antenv
    mod = types.ModuleType("antenv.axon_hooks")
    mod.get_axon_ntff_profile_hook = lambda: _hook
    mod.set_axon_ntff_profile_hook = lambda h: None
    sys.modules["antenv.axon_hooks"] = mod
    antenv.axon_hooks = mod


def profile_once(inputs):
    """Run once with NTFF tracing; return max per-core exec time in ns."""
    import concourse.bass_utils as bu
    bu.upload_artifacts = lambda tmpdir: ""   # no bucket access here
    _install_ntff_hook()
    p = int(inputs["signal_token_num"])
    nc = build_core_kernel(p)
    in_maps = _prep_inputs(
        inputs["x"], inputs["cos"], inputs["sin"], inputs["wq"], inputs["wk"],
        inputs["wv"], inputs["wo"], inputs["q_gamma"], inputs["k_gamma"], p)
    try:
        res = bu.run_bass_kernel_spmd(nc, in_maps, list(range(N_CORES)),
                                      trace=True,
                                      trace_cores=list(range(N_CORES)))
        return res.exec_time_ns
    except Exception as e:
        print(f"profile failed: {type(e).__name__}: {e}")
        return None


# revision 18
# speedup vs baseline: 1.7025x; 1.0569x over previous
"""Trainium2 Bass kernel for GQA attention with QK-RMSNorm, RoPE and a
bidirectional-prefix + causal mask (sparse_attention problem).

Reference computation (fp32):
  xq = x @ wq.T; xk = x @ wk.T; xv = x @ wv.T   (per-head RMSNorm on q,k)
  rope(q), rope(k); repeat kv heads 8x
  scores = q k^T / sqrt(128); mask = causal OR (i<p & j<p)
  out = softmax(scores) @ v;  y = out @ wo.T

Sharding: 8 cores = 2 batches x 4 head-groups (4 query heads each, sharing
one KV head).  Each core computes a partial y^T (its 4 heads' contribution);
the host sums the 4 partials per batch and transposes back.

v2 design notes (vs the first working version):
  * No fp32->fp32r staging copies: matmul-consumed DRAM tensors and SBUF
    tiles are declared float32r (same 32-bit layout host-side) so DMA
    lands them directly; engine-written operands write f32r natively.
  * RMSNorm sum-of-squares runs on the Scalar engine (Square + accum_out),
    the per-token 1/sqrt scale is applied FOR FREE by the PE transpose:
    instead of transposing with the identity, we transpose with
    diag(rq) so qT = q.T @ diag(rq) lands pre-scaled.
  * RoPE sign is folded into sin (host negates the low half), so rope is
    3 wide DVE multiplies + 1 add across all 4 heads at once, reading the
    projection results directly from PSUM.
  * K-path rope + the diag(rq) builds run on the otherwise-idle GPSIMD.
  * Softmax: exp on ACT (per 2-block pair), row sums via a ones-matmul,
    reciprocal on DVE, and the [1,512] -> [128,512] broadcast is a rank-1
    PE matmul (ones outer product) instead of a DRAM round-trip.
  * Single PE instruction stream ordered so the PE never has a long gap
    (projection MMs -> previous tb's transposes -> ... -> attention),
    keeping the HAM clock-gate at 8/8.
  * ACT program order is strictly {Square,Copy,Sqrt} then {Exp,Copy}, so
    exactly two activation-table loads happen.

TRN2 ISA allows ONE sync-wait per instruction and walrus does not split
multi-wait instructions, so `_legalize_waits` rewrites the emitted BIR,
moving excess waits onto preceding same-engine NoOps.
"""
import math
import numpy as np
from contextlib import ExitStack

import bass_rust
import concourse.bass as bass
import concourse.mybir as mybir
import concourse.tile as tile
from concourse.bass_utils import run_bass_kernel_spmd
from concourse.masks import make_identity

F32 = mybir.dt.float32
F32R = mybir.dt.float32r
BF16 = mybir.dt.bfloat16
AF = mybir.ActivationFunctionType

B, S, D = 2, 2048, 2048
NH, KVH, HD = 16, 2, 128
HPC = 4                      # query heads per core
N_CORES = 8
EPS = 1e-6
SOFT_SCALE = 1.0 / math.sqrt(HD)
NEG = -1.0e30

SB = S // 128                # 16 token blocks
DB = D // 128                # 16 contraction blocks

_lgw_counter = [0]


def _legalize_waits(nc, cap=1):
    """Move all-but-`cap` sync waits of every instruction onto preceding
    same-engine NoOps (TRN2 EVENTS block has a single wait slot)."""
    for fn in nc.m.functions:
        for blk in fn.blocks:
            out = []
            changed = False
            for inst in blk.instructions:
                si = inst.sync_info
                waits = list(si.on_wait) if si is not None and si.on_wait else []
                if len(waits) > cap:
                    changed = True
                    move, keep = waits[:-cap], waits[-cap:]
                    for w in move:
                        n = bass_rust.InstNoOp(name=f"LGW-{_lgw_counter[0]}")
                        _lgw_counter[0] += 1
                        n.engine = inst.engine
                        n.sync_info = mybir.SyncInfo(on_wait=[w], on_update=[])
                        out.append(n)
                    inst.sync_info = mybir.SyncInfo(
                        on_wait=keep, on_update=list(si.on_update or []))
                out.append(inst)
            if changed:
                blk.instructions = out
    return nc


def _ext(rb, p):
    """Key extent attended by query row-block rb (rows rb*128 .. rb*128+127)."""
    lo, hi = rb * 128, (rb + 1) * 128
    if hi <= p:
        return p              # prefix rows attend the full prefix [0, p)
    return hi                 # causal rows attend [0, hi), diag-masked


def build_core_kernel(p, legalize=True):
    """One SPMD program; per-core behavior differs only via input data."""
    nc = bass.Bass()

    xB = nc.dram_tensor("xB", [SB, 128, DB, 128], BF16, kind="ExternalInput")
    wqT = nc.dram_tensor("wqT", [D, HPC * HD], BF16, kind="ExternalInput")
    wkvT = nc.dram_tensor("wkvT", [D, 2 * HD], BF16, kind="ExternalInput")
    woT = nc.dram_tensor("woT", [HPC * HD, D], F32R, kind="ExternalInput")
    cs4 = nc.dram_tensor("cs4", [S, 4, HD], F32, kind="ExternalInput")
    dmask = nc.dram_tensor("dmask", [128, 128], F32, kind="ExternalInput")
    rcp_scr = nc.dram_tensor("rcp_scr", [SB * HPC, 512], F32)
    yT = nc.dram_tensor("yT", [D, S], F32, kind="ExternalOutput")

    with tile.TileContext(nc) as tc, ExitStack() as octx:
        const = octx.enter_context(tc.tile_pool(name="const", bufs=1))
        ident = const.tile([128, 128], F32)
        make_identity(nc, ident)
        dmask_sb = const.tile([128, 128], F32)
        nc.sync.dma_start(out=dmask_sb, in_=dmask[:, :])
        eps_t = const.tile([128, 1], F32)
        nc.vector.memset(eps_t, EPS)
        ones_f = const.tile([128, 1], F32)
        nc.vector.memset(ones_f, 1.0)
        ones_col = const.tile([128, 1], F32R)
        nc.vector.tensor_copy(out=ones_col, in_=ones_f)

        qkv = octx.enter_context(tc.tile_pool(name="qkv", bufs=1))
        qT_all = qkv.tile([128, HPC, S], F32R)        # [hd, h, tok]
        kT_all = qkv.tile([128, S], F32R)             # [hd, tok]
        v_all = qkv.tile([128, SB, HD], F32R)         # [tok(P), tb, hd]

        wpool = octx.enter_context(tc.tile_pool(name="w", bufs=1))
        wq_sb = wpool.tile([128, DB, HPC * HD], BF16)
        wkv_sb = wpool.tile([128, DB, 2 * HD], BF16)
        wo_sb = wpool.tile([128, HPC, D], F32R)

        # weight DMAs, split across two queues so x(tb0) isn't starved:
        # wq alternates sync/scalar; wkv likewise (behind wq); wo on scalar.
        for kb in range(DB):
            eng = nc.sync if kb % 2 == 0 else nc.scalar
            eng.dma_start(out=wq_sb[:, kb, :],
                          in_=wqT[kb * 128:(kb + 1) * 128, :])

        # ---------------- Phase 1: QKV projections + norm/rope -------------
        with tc.tile_pool(name="xp", bufs=3) as xp, \
             tc.tile_pool(name="cs", bufs=2) as cs, \
             tc.tile_pool(name="rp", bufs=2) as rp, \
             tc.tile_pool(name="dg", bufs=2) as dgp, \
             tc.tile_pool(name="qps", bufs=2, space="PSUM") as qps_pool, \
             tc.tile_pool(name="kvps", bufs=2, space="PSUM") as kvps_pool, \
             tc.tile_pool(name="trps", bufs=4, space="PSUM") as trps:

            pend_tr = []          # deferred transpose emissions

            def flush_tr():
                for emit in pend_tr:
                    emit()
                del pend_tr[:]

            for tb in range(SB):
                ts = slice(tb * 128, (tb + 1) * 128)

                xb_h = []
                for half in range(2):
                    xbh = xp.tile([128, 8, 128], BF16, tag=f"xb{half}")
                    nc.sync.dma_start(
                        out=xbh, in_=xB[tb, :, half * 8:(half + 1) * 8, :])
                    xb_h.append(xbh)
                cs_t = cs.tile([128, 4, HD], F32, tag="cs")
                nc.sync.dma_start(out=cs_t, in_=cs4[ts, :, :])
                cq, sq = cs_t[:, 0, :], cs_t[:, 1, :]
                ck, sk = cs_t[:, 2, :], cs_t[:, 3, :]
                if tb == 0:
                    for kb in range(DB):
                        eng = nc.sync if kb % 2 == 0 else nc.scalar
                        eng.dma_start(out=wkv_sb[:, kb, :],
                                      in_=wkvT[kb * 128:(kb + 1) * 128, :])
                if tb == 1:
                    for hb in range(HPC):
                        nc.scalar.dma_start(
                            out=wo_sb[:, hb, :],
                            in_=woT[hb * 128:(hb + 1) * 128, :])

                q_ps = qps_pool.tile([128, HPC * HD], F32, tag="q")
                kv_ps = kvps_pool.tile([128, 2 * HD], F32, tag="kv")
                for kb in range(DB):
                    xb = xb_h[kb // 8][:, kb % 8, :]
                    nc.tensor.matmul(q_ps, lhsT=xb, rhs=wq_sb[:, kb, :],
                                     start=(kb == 0), stop=(kb == DB - 1))
                    nc.tensor.matmul(kv_ps, lhsT=xb, rhs=wkv_sb[:, kb, :],
                                     start=(kb == 0), stop=(kb == DB - 1))
                # previous tb's transposes keep the PE busy while this tb's
                # rope/rms runs on DVE/ACT/GPSIMD.
                flush_tr()

                # --- RMS stats on ACT (Square accumulates sum along free) ---
                ms = rp.tile([128, 8], F32, tag="ms")
                scr = rp.tile([128, HD], F32, tag="scr")
                for h in range(HPC):
                    nc.scalar.activation(
                        out=scr, in_=q_ps[:, h * HD:(h + 1) * HD],
                        func=AF.Square, accum_out=ms[:, h:h + 1])
                k_sb = rp.tile([128, HD], F32, tag="k_sb")
                nc.scalar.copy(out=k_sb, in_=kv_ps[:, 0:HD])
                nc.scalar.copy(out=v_all[:, tb, :], in_=kv_ps[:, HD:])
                nc.scalar.activation(out=scr, in_=k_sb, func=AF.Square,
                                     accum_out=ms[:, HPC:HPC + 1])
                srq = rp.tile([128, 8], F32, tag="srq")
                nc.scalar.activation(out=srq[:, 0:HPC + 1],
                                     in_=ms[:, 0:HPC + 1], func=AF.Sqrt,
                                     bias=eps_t, scale=1.0 / HD)
                rq = rp.tile([128, 8], F32, tag="rq")
                nc.vector.reciprocal(out=rq[:, 0:HPC + 1],
                                     in_=srq[:, 0:HPC + 1])

                # --- rope q: wide over all 4 heads, straight from PSUM ---
                # cos/sin broadcast across heads via stride-0 views
                h2 = HD // 2
                cq4 = bass.AP(tensor=cq.tensor, offset=cq.offset,
                              ap=[list(cq.ap[0]), [0, HPC]] + list(cq.ap[1:]))
                sq4 = bass.AP(tensor=sq.tensor, offset=sq.offset,
                              ap=[list(sq.ap[0]), [0, HPC]] + list(sq.ap[1:]))
                q4 = q_ps[:, :].rearrange("pp (h d) -> pp h d", h=HPC)
                t14 = rp.tile([128, HPC, HD], F32, tag="t14")
                nc.vector.tensor_mul(t14, q4, cq4)
                t2p4 = rp.tile([128, HPC, HD], F32, tag="t2p4")
                # low half: q_hi * (-sin_lo)  (sign folded on host)
                nc.vector.tensor_mul(
                    t2p4[:, :, 0:h2], q4[:, :, h2:HD], sq4[:, :, 0:h2])
                nc.vector.tensor_mul(
                    t2p4[:, :, h2:HD], q4[:, :, 0:h2], sq4[:, :, h2:HD])
                qr = rp.tile([128, HPC, HD], F32R, tag="qr")
                nc.vector.tensor_add(qr, t14, t2p4)

                # --- rope k on GPSIMD (SBUF-only engine) ---
                kr = rp.tile([128, HD], F32R, tag="kr")
                t1k = rp.tile([128, HD], F32, tag="t1k")
                nc.gpsimd.tensor_mul(t1k, k_sb, ck)
                t2k = rp.tile([128, HD], F32, tag="t2k")
                nc.gpsimd.tensor_mul(t2k[:, 0:h2], k_sb[:, h2:HD], sk[:, 0:h2])
                nc.gpsimd.tensor_mul(t2k[:, h2:HD], k_sb[:, 0:h2], sk[:, h2:HD])
                nc.gpsimd.tensor_add(kr, t1k, t2k)

                # --- diag(rq) tiles on GPSIMD ---
                dgs = []
                for j in range(HPC + 1):
                    dg = dgp.tile([128, 128], F32R, tag=f"dg{j}")
                    nc.vector.tensor_scalar_mul(dg, ident, rq[:, j:j + 1])
                    dgs.append(dg)

                # --- transposes (deferred into next tb's PE slot):
                #     qT = qr.T @ diag(rq) applies the RMS scale for free ---
                def emit_tr(tb=tb, ts=ts, qr=qr, kr=kr, dgs=dgs):
                    for h in range(HPC):
                        tr_ps = trps.tile([128, 128], F32, tag="tr")
                        nc.tensor.matmul(tr_ps, lhsT=qr[:, h, :],
                                         rhs=dgs[h], start=True, stop=True)
                        if h >= 2:
                            nc.scalar.copy(out=qT_all[:, h, ts], in_=tr_ps)
                        else:
                            nc.vector.tensor_copy(out=qT_all[:, h, ts],
                                                  in_=tr_ps)
                    tr_ps = trps.tile([128, 128], F32, tag="tr")
                    nc.tensor.matmul(tr_ps, lhsT=kr, rhs=dgs[HPC],
                                     start=True, stop=True)
                    nc.vector.tensor_copy(out=kT_all[:, ts], in_=tr_ps)
                pend_tr.append(emit_tr)
            flush_tr()

        # -------- Phase 2+3: attention (group-major) fused with WO ---------
        # Deferred-emission software pipeline, one global pair-iteration
        # counter.  Per pair: scores MMs -> ACT exp -> (post-exp masking on
        # the SBUF exp tile, so no DVE op ever gates the exp) ; sums/AV
        # matmuls drain TWO pair-iterations later so the PE never waits on
        # the ACT exp round-trip.  Softmax normalization runs entirely on
        # ACT + DMA: 1/Z = exp(-ln Z) with a DRAM round-trip broadcast
        # (Ln and Exp share one activation table set).  WO for group g is
        # emitted after head (g+1, 0), giving the last head's
        # normalization a full head of slack; its y tiles go out via DMA
        # straight from PSUM.  PSUM: scores/y 3x2 banks + sums 1 + AV 1.
        n_groups = SB // 4
        with tc.tile_pool(name="exp", bufs=3) as exp_pool, \
             tc.tile_pool(name="attn", bufs=2) as attn_pool, \
             tc.tile_pool(name="avsb", bufs=4) as avsb_pool, \
             tc.tile_pool(name="rcpp", bufs=2) as rcp_pool, \
             tc.tile_pool(name="s_ps", bufs=3, space="PSUM") as s_ps_pool, \
             tc.tile_pool(name="sm_ps", bufs=1, space="PSUM") as sm_ps_pool, \
             tc.tile_pool(name="av_ps", bufs=1, space="PSUM") as av_ps_pool:

            it = [0]              # global pair-iteration counter
            sum_q = []            # (emit_fn,) sums/AV, drained at depth 2
            fin_q = []            # (ready_iter, emit_fn) deferred finishers
            wo_q = []             # deferred WO group emissions

            def drain(keep_sums=2):
                # sums first: a finisher reads av/sm tiles, so the deferred
                # matmuls that write them must be emitted before it
                while len(sum_q) > keep_sums:
                    sum_q.pop(0)()
                while fin_q and fin_q[0][0] <= it[0]:
                    fin_q.pop(0)[1]()

            for g in range(n_groups):
                rbs = list(range(g * 4, g * 4 + 4))
                eblks = [_ext(rb, p) // 128 for rb in rbs]
                gmax = max(eblks)
                qsl = slice(g * 512, (g + 1) * 512)
                attnT = attn_pool.tile([128, HPC, 512], F32R, tag="attnT")

                for h in range(HPC):
                    if h >= 1:
                        for _ in range(min(6, len(wo_q))):
                            wo_q.pop(0)()   # WO(g-1) dbs, spread over heads
                    sm_ps = sm_ps_pool.tile([1, 512], F32, tag="sm")
                    av_ps = av_ps_pool.tile([128, 512], F32, tag="av")

                    for kbp in range(0, gmax, 2):
                        npair = min(2, gmax - kbp)
                        s_ps = s_ps_pool.tile([128, 1024], F32, tag="s")
                        for j in range(npair):
                            kb = kbp + j
                            nc.tensor.matmul(
                                s_ps[:, j * 512:j * 512 + 512],
                                lhsT=kT_all[:, kb * 128:(kb + 1) * 128],
                                rhs=qT_all[:, h, qsl],
                                start=True, stop=True)
                        ex = exp_pool.tile([128, 2, 512], F32R, tag="ex")
                        nc.scalar.activation(
                            out=ex[:, 0:npair, :],
                            in_=s_ps[:, 0:npair * 512],
                            func=AF.Exp, scale=SOFT_SCALE)
                        # post-exp masking on SBUF (never gates the exp):
                        # zero q-columns whose extent <= kb (a prefix of the
                        # group) and multiply the causal diagonal block by
                        # the 0/1 lower-triangle mask.
                        for j in range(npair):
                            kb = kbp + j
                            jm = sum(1 for e in eblks if e <= kb)
                            if jm > 0:
                                # memset can't write f32r; scale by 0 instead
                                nc.vector.tensor_scalar_mul(
                                    ex[:, j, 0:jm * 128],
                                    ex[:, j, 0:jm * 128], 0.0)
                            ri_d = kb - g * 4
                            if 0 <= ri_d < 4 and kb * 128 >= p \
                                    and eblks[ri_d] == kb + 1:
                                od = ri_d * 128
                                nc.vector.tensor_mul(
                                    ex[:, j, od:od + 128],
                                    ex[:, j, od:od + 128], dmask_sb)

                        def emit_sums(ex=ex, npair=npair, kbp=kbp,
                                      gmax=gmax, sm_ps=sm_ps, av_ps=av_ps):
                            for j in range(npair):
                                kb = kbp + j
                                exj = ex[:, j, :]
                                nc.tensor.matmul(sm_ps, lhsT=ones_col,
                                                 rhs=exj, start=(kb == 0),
                                                 stop=(kb == gmax - 1))
                                nc.tensor.matmul(av_ps,
                                                 lhsT=v_all[:, kb, :],
                                                 rhs=exj, start=(kb == 0),
                                                 stop=(kb == gmax - 1))
                        sum_q.append(emit_sums)
                        it[0] += 1
                        drain(keep_sums=2)

                    # head finishers, deferred 2 pair-iterations:
                    #   stage A: av -> SBUF (ACT), lnZ (ACT), lnZ -> DRAM
                    #   stage B (2 more iters): bcast read, 1/Z = exp(-lnZ)
                    #            (ACT), attnT = av * 1/Z (DVE)
                    slot = g * HPC + h
                    def fin_a(slot=slot, sm_ps=sm_ps, av_ps=av_ps,
                              h=h, attnT=attnT):
                        lnz = rcp_pool.tile([1, 512], F32, tag="lnz")
                        nc.scalar.activation(out=lnz, in_=sm_ps, func=AF.Ln)
                        nc.sync.dma_start(out=rcp_scr[slot:slot + 1, :],
                                          in_=lnz)
                        av_sb = avsb_pool.tile([128, 512], F32, tag="av_sb")
                        nc.vector.tensor_copy(out=av_sb, in_=av_ps)
                        def fin_b(slot=slot, av_sb=av_sb, h=h, attnT=attnT):
                            lbc = rcp_pool.tile([128, 512], F32, tag="lbc")
                            drap = rcp_scr[slot:slot + 1, :]
                            bcast = bass.AP(tensor=drap.tensor,
                                            offset=drap.offset,
                                            ap=[[0, 128]] + list(drap.ap[1:]))
                            nc.sync.dma_start(out=lbc, in_=bcast)
                            rbc = rcp_pool.tile([128, 512], F32, tag="rbc")
                            nc.scalar.activation(out=rbc, in_=lbc,
                                                 func=AF.Exp, scale=-1.0)
                            nc.vector.tensor_mul(attnT[:, h, :], av_sb, rbc)
                        fin_q.append((it[0] + 2, fin_b))
                    fin_q.append((it[0] + 2, fin_a))

                # ---- WO for this token chunk, deferred one head and
                #      spread across the next group's heads ----
                def emit_wo_db(g=g, attnT=attnT, db=0):
                    y_ps = s_ps_pool.tile([128, 1024], F32, tag="s")
                    for hb in range(HPC):
                        nc.tensor.matmul(
                            y_ps[:, 0:512],
                            lhsT=wo_sb[:, hb, db * 128:(db + 1) * 128],
                            rhs=attnT[:, hb, :],
                            start=(hb == 0), stop=(hb == HPC - 1))
                    y_sb = avsb_pool.tile([128, 512], F32, tag="y_sb")
                    nc.vector.tensor_copy(out=y_sb, in_=y_ps[:, 0:512])
                    nc.sync.dma_start(
                        out=yT[db * 128:(db + 1) * 128,
                               g * 512:(g + 1) * 512],
                        in_=y_sb)
                for db in range(DB):
                    wo_q.append(lambda g=g, attnT=attnT, db=db:
                                emit_wo_db(g, attnT, db))

            while sum_q or fin_q:
                it[0] += 1
                drain(keep_sums=0)
            while wo_q:
                wo_q.pop(0)()

    if legalize:
        _legalize_waits(nc)
    return nc


def _prep_inputs(x, cos, sin, wq, wk, wv, wo, q_gamma, k_gamma, p):
    """Build the 8 per-core input maps."""
    cos2 = np.asarray(cos, np.float32).reshape(S, HD)
    sin2 = np.asarray(sin, np.float32).reshape(S, HD)
    qg = np.asarray(q_gamma, np.float32)
    kg = np.asarray(k_gamma, np.float32)
    h = HD // 2
    qg_rot = np.concatenate([qg[h:], qg[:h]])
    kg_rot = np.concatenate([kg[h:], kg[:h]])
    cos_q = cos2 * qg
    sin_q = sin2 * qg_rot
    cos_k = cos2 * kg
    sin_k = sin2 * kg_rot
    # fold the rotate-half sign into the low halves of sin
    sin_q[:, :h] *= -1.0
    sin_k[:, :h] *= -1.0
    # pack [cos_q | sin_q | cos_k | sin_k] so each tb is one 2KB-row DMA
    cs4 = np.ascontiguousarray(
        np.stack([cos_q, sin_q, cos_k, sin_k], axis=1))

    ii = np.arange(128)
    dmask = (ii[:, None] <= ii[None, :]).astype(np.float32)

    x = np.asarray(x, np.float32)
    wq = np.asarray(wq, np.float32)
    wk = np.asarray(wk, np.float32)
    wv = np.asarray(wv, np.float32)
    wo = np.asarray(wo, np.float32)

    # xP[tb, pp, kb, ti] = x[b, tb*128+ti, kb*128+pp]: the per-tb SBUF
    # x tile loads become 128 contiguous 4KB descriptors instead of 1024
    # 512B ones.
    import ml_dtypes
    xB = [np.ascontiguousarray(
        x[b].reshape(SB, 128, DB, 128).transpose(0, 3, 2, 1)
        .astype(ml_dtypes.bfloat16)) for b in range(B)]
    in_maps = []
    for c in range(N_CORES):
        b, g = divmod(c, N_CORES // B)
        h0 = g * HPC
        kv = h0 // (NH // KVH)
        wqTc = np.ascontiguousarray(
            wq[h0 * HD:(h0 + HPC) * HD, :].T).astype(ml_dtypes.bfloat16)
        import ml_dtypes
        wkvTc = np.ascontiguousarray(
            np.concatenate([wk[kv * HD:(kv + 1) * HD, :],
                            wv[kv * HD:(kv + 1) * HD, :]],
                           axis=0).T).astype(ml_dtypes.bfloat16)
        woTc = np.ascontiguousarray(wo[:, h0 * HD:(h0 + HPC) * HD].T)
        in_maps.append({
            "xB": xB[b], "wqT": wqTc, "wkvT": wkvTc,
            "woT": woTc, "cs4": cs4, "dmask": dmask,
        })
    return in_maps


def _gather(results):
    y = np.zeros((B, S, D), dtype=np.float32)
    for c in range(N_CORES):
        b = c // (N_CORES // B)
        y[b] += results[c]["yT"].T
    return y


def kernel(x, cos, sin, wq, wk, wv, wo, q_gamma, k_gamma, signal_token_num):
    p = int(signal_token_num)
    assert p % 128 == 0 and 0 <= p <= S, f"unsupported signal_token_num {p}"

    nc = build_core_kernel(p)
    in_maps = _prep_inputs(x, cos, sin, wq, wk, wv, wo, q_gamma, k_gamma, p)
    res = run_bass_kernel_spmd(nc, in_maps, list(range(N_CORES)))
    return _gather(res.results)


def _install_ntff_hook():
    """The container's antenv lacks axon_hooks; replicate the boot-time NTFF
    profile hook (ctypes into libaxon_pjrt.so) and register the module."""
    import sys
    import types
    import ctypes
    import contextlib

    if "antenv.axon_hooks" in sys.modules:
        return
    so_path = "/opt/axon/libaxon_pjrt.so"
    lib = ctypes.CDLL(so_path)
    if not hasattr(lib, "axon_start_nrt_profile"):
        return
    lib.axon_start_nrt_profile.argtypes = [
        ctypes.POINTER(ctypes.c_int64), ctypes.c_size_t]
    lib.axon_start_nrt_profile.restype = ctypes.c_int64
    lib.axon_stop_nrt_profile.argtypes = [ctypes.c_char_p]
    lib.axon_stop_nrt_profile.restype = ctypes.c_int64

    @contextlib.contextmanager
    def _hook(output_dir, device_ids):
        import jax
        jax.devices()
        if device_ids:
            ids = (ctypes.c_int64 * len(device_ids))(*device_ids)
            rc = lib.axon_start_nrt_profile(ids, len(device_ids))
        else:
            rc = lib.axon_start_nrt_profile(None, 0)
        if rc != 0:
            raise RuntimeError(f"axon_start_nrt_profile rc={rc}")
        try:
            yield
        finally:
            n = lib.axon_stop_nrt_profile(str(output_dir).encode())
            print(f"profile: {n} file(s) written to {output_dir}")

    import antenv
    mod = types.ModuleType("antenv.axon_hooks")
    mod.get_axon_ntff_profile_hook = lambda: _hook
    mod.set_axon_ntff_profile_hook = lambda h: None
    sys.modules["antenv.axon_hooks"] = mod
    antenv.axon_hooks = mod


def profile_once(inputs):
    """Run once with NTFF tracing; return max per-core exec time in ns."""
    import concourse.bass_utils as bu
    bu.upload_artifacts = lambda tmpdir: ""   # no bucket access here
    _install_ntff_hook()
    p = int(inputs["signal_token_num"])
    nc = build_core_kernel(p)
    in_maps = _prep_inputs(
        inputs["x"], inputs["cos"], inputs["sin"], inputs["wq"], inputs["wk"],
        inputs["wv"], inputs["wo"], inputs["q_gamma"], inputs["k_gamma"], p)
    try:
        res = bu.run_bass_kernel_spmd(nc, in_maps, list(range(N_CORES)),
                                      trace=True,
                                      trace_cores=list(range(N_CORES)))
        return res.exec_time_ns
    except Exception as e:
        print(f"profile failed: {type(e).__name__}: {e}")
        return None
